# revision 18
# baseline (speedup 1.0000x reference)
"""Trainium2 Bass kernel for the nn_Attention problem.

Cross-attention transformer block: QKV projections, masked softmax
attention with a post-softmax query-mask multiply, concat + final linear,
residual, LayerNorm.  Returns (x, attns) like the reference.

Sharding: data-parallel over batch B=8 across the 8 NeuronCores — each
core computes one batch element end-to-end; no collectives.

Device-side layout trick: all attention tensors are kept "transposed"
(contraction dim on partitions) so every matmul lhsT/rhs is a natural
slice — the host pre-transposes decoder/memory/mask per core, and the
attention weights output is produced as expT [k, q] + row-normalizers r,
with the final attns = (expT.T * r) assembled on the host.
"""
import sys, os

for p in ("/opt/trn_rl_repo",):
    if p not in sys.path and os.path.isdir(p):
        sys.path.insert(0, p)

import numpy as np
import ml_dtypes

import concourse.bass as bass
import concourse.mybir as mybir
import concourse.tile as tile
from concourse import bacc
from concourse.bass_utils import run_bass_kernel_spmd

F32 = mybir.dt.float32
BF16 = mybir.dt.bfloat16
BF = ml_dtypes.bfloat16

B, S, D, H, P = 8, 1024, 1024, 8, 128
DC = D // P          # 8 contraction chunks of 128
SH = 512             # free-dim half
NEG = float(-2**32 + 1)
SCALE = float(1.0 / np.sqrt(P))  # 1/sqrt(dh), dh = 128
LN_EPS = 1e-5

_CACHE = {}


def build():
    nc = bacc.Bacc(None, target_bir_lowering=False)

    # ---- per-core inputs (bf16 matmul operands staged by host) ----
    decT = nc.dram_tensor("decT", [P, DC, S], BF16, kind="ExternalInput")
    memT = nc.dram_tensor("memT", [P, DC, S], BF16, kind="ExternalInput")
    keepT = nc.dram_tensor("keepT", [P, DC, S], BF16, kind="ExternalInput")
    dec_res = nc.dram_tensor("dec_res", [S, D], F32, kind="ExternalInput")
    qmask = nc.dram_tensor("qmask", [1, S], F32, kind="ExternalInput")
    # ---- shared weights / constants ----
    wq = nc.dram_tensor("wq", [P, DC, D], BF16, kind="ExternalInput")
    wk = nc.dram_tensor("wk", [P, DC, D], BF16, kind="ExternalInput")
    wv = nc.dram_tensor("wv", [P, DC, D], BF16, kind="ExternalInput")
    wf_top = nc.dram_tensor("wf_top", [P, DC, D], BF16, kind="ExternalInput")
    wf_bot = nc.dram_tensor("wf_bot", [P, DC, D], BF16, kind="ExternalInput")
    ones_c = nc.dram_tensor("ones_c", [P, 1], BF16, kind="ExternalInput")
    ones_r = nc.dram_tensor("ones_r", [1, P], BF16, kind="ExternalInput")

    # ---- outputs ----
    x_out = nc.dram_tensor("x_out", [S, D], F32, kind="ExternalOutput")
    expt = nc.dram_tensor("expt", [H, S, S], BF16, kind="ExternalOutput")
    r_out = nc.dram_tensor("r_out", [H, S], F32, kind="ExternalOutput")

    with tile.TileContext(nc) as tc:
        with (
            tc.tile_pool(name="sb", bufs=1) as sb,
            tc.tile_pool(name="ps", bufs=1, space="PSUM") as ps,
        ):
            # ---------- constants ----------
            t_ones_c = sb.tile([P, 1], BF16)
            t_ones_r = sb.tile([1, P], BF16)
            t_qmask = sb.tile([1, S], F32)
            t_eps = sb.tile([P, 1], F32)
            nc.vector.memset(t_eps[:], LN_EPS)

            # ---------- persistent big tiles ----------
            # decT/wq as per-chunk tiles so the first projection matmuls can
            # start as soon as chunk 0 lands (deps are tracked per tile)
            t_decT_c = [sb.tile([P, S], BF16, name=f"t_decT_{dc}")
                        for dc in range(DC)]
            t_keepT = sb.tile([P, DC, S], BF16)
            # (keepT DMA deferred below so early-phase DMAs get the bandwidth)
            t_qt = sb.tile([P, H, S], BF16)      # Q^T  [dh, head, q]
            t_kt = sb.tile([P, H, S], BF16)      # K^T  [dh, head, k]
            t_v = sb.tile([P, DC, D], BF16)      # V    [k_inner, k_chunk, dout]
            t_xoutT = sb.tile([P, H, S], BF16)   # attn_out^T [dh, head, q]
            for dc in range(DC):
                eng = nc.sync if dc % 2 == 0 else nc.scalar
                eng.dma_start(t_decT_c[dc][:], decT[:, dc, :])

            # ============ Phase A: projections ============
            # memT shares a rotation tag with the per-head expT tiles.
            t_memT = sb.tile([P, DC, S], BF16, tag="big_rot", bufs=2,
                             name="t_memT")
            for dc in range(DC):
                nc.sync.dma_start(t_memT[:, dc, :], memT[:, dc, :])
            t_wq_c = [sb.tile([P, D], BF16, tag="wqc_eraw", bufs=8,
                                name=f"t_wq_{dc}")
                      for dc in range(DC)]
            t_wk = sb.tile([P, DC, D], BF16, tag="w", bufs=2, name="t_wk")
            for dc in range(DC):
                eng = nc.scalar if dc % 2 == 0 else nc.sync
                eng.dma_start(t_wq_c[dc][:], wq[:, dc, :])
            for dc in range(DC):
                nc.sync.dma_start(t_wk[:, dc, :], wk[:, dc, :])

            def proj(dst, w_sl, rhs_sl):
                # dst[:, m, qh] = sum_dc w[dc][:, m-tile].T @ rhs[dc][:, qh]
                for qh in range(2):
                    for m in range(H):
                        pt = ps.tile([P, SH], F32, tag="mm", bufs=2,
                                     name=f"p_mm_{m}_{qh}")
                        for dc in range(DC):
                            nc.tensor.matmul(
                                pt[:],
                                w_sl(dc)[:, m * P:(m + 1) * P],
                                rhs_sl(dc)[:, qh * SH:(qh + 1) * SH],
                                start=(dc == 0), stop=(dc == DC - 1))
                        nc.vector.tensor_copy(
                            dst[:, m, qh * SH:(qh + 1) * SH], pt[:])

            proj(t_qt, lambda dc: t_wq_c[dc][:], lambda dc: t_decT_c[dc][:])

            for dc in range(DC):
                nc.sync.dma_start(t_keepT[:, dc, :], keepT[:, dc, :])
            nc.sync.dma_start(t_ones_c[:], ones_c[:])
            nc.sync.dma_start(t_ones_r[:], ones_r[:])
            nc.sync.dma_start(t_qmask[:], qmask[:])

            proj(t_kt, lambda dc: t_wk[:, dc, :], lambda dc: t_memT[:, dc, :])

            t_wv = sb.tile([P, DC, D], BF16, tag="w", bufs=2, name="t_wv")
            for dc in range(DC):
                nc.sync.dma_start(t_wv[:, dc, :], wv[:, dc, :])
            # V native: psum[k-tile, dout-half] = sum_dc memT[:,dc,ktile].T @ wv[:,dc,dh]
            for dh in range(2):
                for kt in range(H):
                    pt = ps.tile([P, SH], F32, tag="mm", bufs=2,
                                 name=f"p_v_{kt}_{dh}")
                    for dc in range(DC):
                        nc.tensor.matmul(
                            pt[:],
                            t_memT[:, dc, kt * P:(kt + 1) * P],
                            t_wv[:, dc, dh * SH:(dh + 1) * SH],
                            start=(dc == 0), stop=(dc == DC - 1))
                    nc.vector.tensor_copy(
                        t_v[:, kt, dh * SH:(dh + 1) * SH], pt[:])

            # ============ Phase B: attention per head ============
            def emit_rep(j, t_rbf, t_rrep, xparts):
                # replicate r across partitions, then scale this head's
                # attn@V psum tiles into xoutT (deferred to overlap with the
                # next head's scores so the PE never stalls on the DVE chain)
                for qh in range(2):
                    pt = ps.tile([P, SH], F32, tag="rep", bufs=1,
                                 name=f"p_rep_{j}_{qh}")
                    nc.tensor.matmul(pt[:], t_ones_r[:],
                                     t_rbf[:, qh * SH:(qh + 1) * SH],
                                     start=True, stop=True)
                    nc.vector.tensor_copy(t_rrep[:, qh * SH:(qh + 1) * SH],
                                          pt[:])
                for qh in range(2):
                    nc.vector.tensor_mul(
                        t_xoutT[:, j, qh * SH:(qh + 1) * SH], xparts[qh],
                        t_rrep[:, qh * SH:(qh + 1) * SH])

            def emit_scores(j):
                t_exp = sb.tile([P, DC, S], BF16, tag="big_rot", bufs=2,
                                name=f"t_exp_{j}")
                for i in range(DC):
                    pt = ps.tile([P, 2, SH], F32, tag="s", bufs=2,
                                 name=f"p_s_{j}_{i}")
                    for qh in range(2):
                        nc.tensor.matmul(
                            pt[:, qh, :],
                            t_kt[:, j, i * P:(i + 1) * P],
                            t_qt[:, j, qh * SH:(qh + 1) * SH],
                            start=True, stop=True)
                    t_eraw = sb.tile([P, S], BF16, tag="wqc_eraw", bufs=8,
                                     name=f"t_eraw_{j}_{i}")
                    nc.scalar.activation(
                        out=t_eraw[:],
                        in_=pt[:].rearrange("p a b -> p (a b)"),
                        func=mybir.ActivationFunctionType.Exp, scale=SCALE)
                    mask_eng = nc.vector if i % 2 == 0 else nc.gpsimd
                    mask_eng.tensor_mul(
                        t_exp[:, i, :], t_eraw[:], t_keepT[:, i, :])
                    nc.sync.dma_start(expt[j, i * P:(i + 1) * P, :],
                                      t_exp[:, i, :])
                return t_exp

            def emit_reduce(j, t_exp):
                # softmax denominators: ones^T @ expT -> [1, q]; then attn@V
                t_rrow = sb.tile([1, S], F32, tag="rrow", bufs=1,
                                 name=f"t_rrow_{j}")
                for qh in range(2):
                    pt = ps.tile([1, SH], F32, tag="sum", bufs=1,
                                 name=f"p_sum_{j}_{qh}")
                    for i in range(DC):
                        nc.tensor.matmul(
                            pt[:], t_ones_c[:],
                            t_exp[:, i, qh * SH:(qh + 1) * SH],
                            start=(i == 0), stop=(i == DC - 1))
                    nc.vector.reciprocal(t_rrow[:, qh * SH:(qh + 1) * SH],
                                         pt[:])
                nc.vector.tensor_mul(t_rrow[:], t_rrow[:], t_qmask[:])
                nc.sync.dma_start(r_out[j:j + 1, :], t_rrow[:])
                t_rbf = sb.tile([1, S], BF16, tag="rbf", bufs=1,
                                name=f"t_rbf_{j}")
                nc.vector.tensor_copy(t_rbf[:], t_rrow[:])

                xparts = []
                for qh in range(2):
                    pt = ps.tile([P, SH], F32, tag="mm", bufs=2,
                                 name=f"p_x_{j}_{qh}")
                    for i in range(DC):
                        nc.tensor.matmul(
                            pt[:],
                            t_v[:, i, j * P:(j + 1) * P],
                            t_exp[:, i, qh * SH:(qh + 1) * SH],
                            start=(i == 0), stop=(i == DC - 1))
                    xparts.append(pt)

                # replicate r across partitions; scale attn@V into xoutT
                t_rrep = sb.tile([P, S], F32, tag="rrep", bufs=1,
                                 name=f"t_rrep_{j}")
                for qh in range(2):
                    pt = ps.tile([P, SH], F32, tag="rep", bufs=1,
                                 name=f"p_rep_{j}_{qh}")
                    nc.tensor.matmul(pt[:], t_ones_r[:],
                                     t_rbf[:, qh * SH:(qh + 1) * SH],
                                     start=True, stop=True)
                    nc.vector.tensor_copy(t_rrep[:, qh * SH:(qh + 1) * SH],
                                          pt[:])
                for qh in range(2):
                    nc.vector.tensor_mul(
                        t_xoutT[:, j, qh * SH:(qh + 1) * SH], xparts[qh],
                        t_rrep[:, qh * SH:(qh + 1) * SH])

            prev = None
            for j in range(H):
                t_exp = emit_scores(j)
                if prev is not None:
                    emit_reduce(prev[0], prev[1])
                prev = (j, t_exp)
            emit_reduce(prev[0], prev[1])

            # ============ Phase C: final linear + residual + LayerNorm ============
            t_wft = sb.tile([P, DC, D], BF16, tag="w", bufs=2, name="t_wft")
            t_wfb = sb.tile([P, DC, D], BF16, tag="w", bufs=2, name="t_wfb")
            for dc in range(DC):
                nc.sync.dma_start(t_wft[:, dc, :], wf_top[:, dc, :])
            for dc in range(DC):
                nc.sync.dma_start(t_wfb[:, dc, :], wf_bot[:, dc, :])

            for t in range(H):
                t_dec = sb.tile([P, D], F32, tag="dec", bufs=2,
                                name=f"t_dec_{t}")
                nc.sync.dma_start(t_dec[:], dec_res[t * P:(t + 1) * P, :])
                t_x2 = sb.tile([P, D], F32, tag="x2", bufs=2, name=f"t_x2_{t}")
                for n in range(2):
                    pt = ps.tile([P, SH], F32, tag="mm", bufs=2,
                                 name=f"p_f_{t}_{n}")
                    for dc in range(DC):
                        nc.tensor.matmul(
                            pt[:],
                            t_decT_c[dc][:, t * P:(t + 1) * P],
                            t_wft[:, dc, n * SH:(n + 1) * SH],
                            start=(dc == 0), stop=False)
                    for j in range(H):
                        nc.tensor.matmul(
                            pt[:],
                            t_xoutT[:, j, t * P:(t + 1) * P],
                            t_wfb[:, j, n * SH:(n + 1) * SH],
                            start=False, stop=(j == H - 1))
                    nc.vector.tensor_add(t_x2[:, n * SH:(n + 1) * SH], pt[:],
                                         t_dec[:, n * SH:(n + 1) * SH])

                # LayerNorm along the free dim (D = 1024, 2 bn_stats subgroups)
                t_stats = sb.tile([P, 2, 6], F32, tag="stats", bufs=2,
                                  name=f"t_stats_{t}")
                for g in range(2):
                    nc.vector.bn_stats(out=t_stats[:, g, :],
                                       in_=t_x2[:, g * SH:(g + 1) * SH])
                t_mv = sb.tile([P, 2], F32, tag="mv", bufs=2, name=f"t_mv_{t}")
                nc.vector.bn_aggr(out=t_mv[:], in_=t_stats[:])
                t_rstd = sb.tile([P, 1], F32, tag="rstd", bufs=2,
                                 name=f"t_rstd_{t}")
                nc.scalar.activation(
                    out=t_rstd[:], in_=t_mv[:, 1:2],
                    func=mybir.ActivationFunctionType.Sqrt,
                    bias=t_eps[:], scale=1.0)
                nc.vector.reciprocal(t_rstd[:], t_rstd[:])
                nc.vector.tensor_scalar(
                    t_x2[:], t_x2[:], scalar1=t_mv[:, 0:1], scalar2=t_rstd[:],
                    op0=mybir.AluOpType.subtract, op1=mybir.AluOpType.mult)
                nc.sync.dma_start(x_out[t * P:(t + 1) * P, :], t_x2[:])

    nc.compile()
    return nc


def _stage_t(arr):
    """[S, D]-like 2D -> [P, DC, S] bf16 with dim0 = (dc, dp) transposed."""
    return np.ascontiguousarray(
        arr.T.reshape(DC, P, -1).transpose(1, 0, 2)).astype(BF)


def _stage_w(w):
    """[D, D] weight -> [P, DC, D] bf16 (contraction rows onto partitions)."""
    return np.ascontiguousarray(
        w.reshape(DC, P, -1).transpose(1, 0, 2)).astype(BF)


def kernel(memory, decoder_input, query_mask, Wk, Wv, Wq, Wf, bf, gamma, beta,
           mask):
    memory = np.asarray(memory, np.float32)
    decoder_input = np.asarray(decoder_input, np.float32)
    query_mask = np.asarray(query_mask, np.float32)
    Wk = np.asarray(Wk, np.float32)
    Wv = np.asarray(Wv, np.float32)
    Wq = np.asarray(Wq, np.float32)
    Wf = np.asarray(Wf, np.float32)
    bf = np.asarray(bf, np.float32)
    gamma = np.asarray(gamma, np.float32)
    beta = np.asarray(beta, np.float32)
    mask = np.asarray(mask)

    if "nc" not in _CACHE:
        _CACHE["nc"] = build()
    nc = _CACHE["nc"]

    shared = dict(
        wq=_stage_w(Wq), wk=_stage_w(Wk), wv=_stage_w(Wv),
        wf_top=_stage_w(Wf[:D]), wf_bot=_stage_w(Wf[D:]),
        ones_c=np.ones((P, 1), BF),
        ones_r=np.ones((1, P), BF),
    )
    in_maps = []
    for b in range(B):
        in_maps.append(dict(
            shared,
            decT=_stage_t(decoder_input[b]),
            memT=_stage_t(memory[b]),
            keepT=np.ascontiguousarray(
                (~mask[b]).T.astype(np.float32)
                .reshape(DC, P, S).transpose(1, 0, 2)).astype(BF),
            dec_res=np.ascontiguousarray(decoder_input[b] + bf[None, :],
                                         dtype=np.float32),
            qmask=np.ascontiguousarray(query_mask[b][None, :],
                                       dtype=np.float32),
        ))

    res = run_bass_kernel_spmd(nc, in_maps, core_ids=list(range(B)),
                               **_CACHE.get("run_kwargs", {}))
    _CACHE["last_result"] = res

    x = np.empty((B, S, D), np.float32)
    attns = np.empty((H * B, S, S), np.float32)
    apply_gb = (not np.all(gamma == 1.0)) or (not np.all(beta == 0.0))
    for b in range(B):
        rb = res.results[b]
        if apply_gb:
            x[b] = rb["x_out"] * gamma[None, :] + beta[None, :]
        else:
            x[b] = rb["x_out"]
        e = rb["expt"].astype(np.float32)          # [H, k, q]
        r = rb["r_out"]                            # [H, q]
        for j in range(H):
            np.multiply(e[j].T, r[j][:, None], out=attns[j * B + b])
    return x, attns


if __name__ == "__main__":
    rng = np.random.default_rng(0)
    ins = dict(
        memory=rng.standard_normal((B, S, D), dtype=np.float32),
        decoder_input=rng.standard_normal((B, S, D), dtype=np.float32),
        query_mask=rng.random((B, S), dtype=np.float32),
        Wk=(rng.standard_normal((D, D), dtype=np.float32) * 0.02),
        Wv=(rng.standard_normal((D, D), dtype=np.float32) * 0.02),
        Wq=(rng.standard_normal((D, D), dtype=np.float32) * 0.02),
        Wf=(rng.standard_normal((2 * D, D), dtype=np.float32) * 0.02),
        bf=np.zeros(D, np.float32),
        gamma=np.ones(D, np.float32),
        beta=np.zeros(D, np.float32),
        mask=rng.integers(0, 2, (B, S, S)) == 1,
    )
    x, attns = kernel(**ins)
    print("ran", x.shape, attns.shape)


# revision 19
# speedup vs baseline: 1.1312x; 1.1312x over previous
"""Trainium2 Bass kernel for the nn_Attention problem.

Cross-attention transformer block: QKV projections, masked softmax
attention with a post-softmax query-mask multiply, concat + final linear,
residual, LayerNorm.  Returns (x, attns) like the reference.

Sharding: data-parallel over batch B=8 across the 8 NeuronCores — each
core computes one batch element end-to-end; no collectives.

Device-side layout trick: all attention tensors are kept "transposed"
(contraction dim on partitions) so every matmul lhsT/rhs is a natural
slice — the host pre-transposes decoder/memory/mask per core, and the
attention weights output is produced as expT [k, q] + row-normalizers r,
with the final attns = (expT.T * r) assembled on the host.
"""
import sys, os

for p in ("/opt/trn_rl_repo",):
    if p not in sys.path and os.path.isdir(p):
        sys.path.insert(0, p)

import numpy as np
import ml_dtypes

import concourse.bass as bass
import concourse.mybir as mybir
import concourse.tile as tile
from concourse import bacc
from concourse.bass_utils import run_bass_kernel_spmd

F32 = mybir.dt.float32
BF16 = mybir.dt.bfloat16
BF = ml_dtypes.bfloat16

B, S, D, H, P = 8, 1024, 1024, 8, 128
DC = D // P          # 8 contraction chunks of 128
SH = 512             # free-dim half
NEG = float(-2**32 + 1)
SCALE = float(1.0 / np.sqrt(P))  # 1/sqrt(dh), dh = 128
LN_EPS = 1e-5

_CACHE = {}


def build():
    nc = bacc.Bacc(None, target_bir_lowering=False)

    # ---- per-core inputs (bf16 matmul operands staged by host) ----
    decT = nc.dram_tensor("decT", [P, DC, S], BF16, kind="ExternalInput")
    memT = nc.dram_tensor("memT", [P, DC, S], BF16, kind="ExternalInput")
    keepT = nc.dram_tensor("keepT", [P, DC, S], BF16, kind="ExternalInput")
    dec_res = nc.dram_tensor("dec_res", [S, D], F32, kind="ExternalInput")
    qmask = nc.dram_tensor("qmask", [P, S], F32, kind="ExternalInput")
    # ---- shared weights / constants ----
    wq = nc.dram_tensor("wq", [P, DC, D], BF16, kind="ExternalInput")
    wk = nc.dram_tensor("wk", [P, DC, D], BF16, kind="ExternalInput")
    wv = nc.dram_tensor("wv", [P, DC, D], BF16, kind="ExternalInput")
    wf_top = nc.dram_tensor("wf_top", [P, DC, D], BF16, kind="ExternalInput")
    wf_bot = nc.dram_tensor("wf_bot", [P, DC, D], BF16, kind="ExternalInput")
    ones_c = nc.dram_tensor("ones_c", [P, P], BF16, kind="ExternalInput")

    # ---- outputs ----
    x_out = nc.dram_tensor("x_out", [S, D], F32, kind="ExternalOutput")
    expt = nc.dram_tensor("expt", [H, S, S], BF16, kind="ExternalOutput")
    r_out = nc.dram_tensor("r_out", [H, S], F32, kind="ExternalOutput")

    with tile.TileContext(nc) as tc:
        with (
            tc.tile_pool(name="sb", bufs=1) as sb,
            tc.tile_pool(name="ps", bufs=1, space="PSUM") as ps,
        ):
            # ---------- constants ----------
            t_ones_c = sb.tile([P, P], BF16)
            t_qmask = sb.tile([P, S], F32)
            t_eps = sb.tile([P, 1], F32)
            nc.vector.memset(t_eps[:], LN_EPS)

            # ---------- persistent big tiles ----------
            # decT/wq as per-chunk tiles so the first projection matmuls can
            # start as soon as chunk 0 lands (deps are tracked per tile)
            t_decT_c = [sb.tile([P, S], BF16, name=f"t_decT_{dc}")
                        for dc in range(DC)]
            t_keepT = sb.tile([P, DC, S], BF16)
            # (keepT DMA deferred below so early-phase DMAs get the bandwidth)
            t_qt = sb.tile([P, H, S], BF16)      # Q^T  [dh, head, q]
            t_kt = sb.tile([P, H, S], BF16)      # K^T  [dh, head, k]
            t_v = sb.tile([P, DC, D], BF16)      # V    [k_inner, k_chunk, dout]
            t_xoutT = sb.tile([P, H, S], BF16)   # attn_out^T [dh, head, q]
            for dc in range(DC):
                eng = nc.sync if dc % 2 == 0 else nc.scalar
                eng.dma_start(t_decT_c[dc][:], decT[:, dc, :])

            # ============ Phase A: projections ============
            # memT shares a rotation tag with the per-head expT tiles.
            t_memT = sb.tile([P, DC, S], BF16, tag="big_rot", bufs=2,
                             name="t_memT")
            for dc in range(DC):
                nc.sync.dma_start(t_memT[:, dc, :], memT[:, dc, :])
            t_wq_c = [sb.tile([P, D], BF16, tag="wqc_eraw", bufs=8,
                                name=f"t_wq_{dc}")
                      for dc in range(DC)]
            t_wk = sb.tile([P, DC, D], BF16, tag="w", bufs=2, name="t_wk")
            for dc in range(DC):
                eng = nc.scalar if dc % 2 == 0 else nc.sync
                eng.dma_start(t_wq_c[dc][:], wq[:, dc, :])
            for dc in range(DC):
                nc.sync.dma_start(t_wk[:, dc, :], wk[:, dc, :])

            def proj(dst, w_sl, rhs_sl):
                # dst[:, m, qh] = sum_dc w[dc][:, m-tile].T @ rhs[dc][:, qh]
                for qh in range(2):
                    for m in range(H):
                        pt = ps.tile([P, SH], F32, tag="mm", bufs=2,
                                     name=f"p_mm_{m}_{qh}")
                        for dc in range(DC):
                            nc.tensor.matmul(
                                pt[:],
                                w_sl(dc)[:, m * P:(m + 1) * P],
                                rhs_sl(dc)[:, qh * SH:(qh + 1) * SH],
                                start=(dc == 0), stop=(dc == DC - 1))
                        nc.vector.tensor_copy(
                            dst[:, m, qh * SH:(qh + 1) * SH], pt[:])

            proj(t_qt, lambda dc: t_wq_c[dc][:], lambda dc: t_decT_c[dc][:])

            for dc in range(DC):
                nc.sync.dma_start(t_keepT[:, dc, :], keepT[:, dc, :])
            nc.sync.dma_start(t_ones_c[:], ones_c[:])
            nc.sync.dma_start(t_qmask[:], qmask[:])

            proj(t_kt, lambda dc: t_wk[:, dc, :], lambda dc: t_memT[:, dc, :])

            t_wv = sb.tile([P, DC, D], BF16, tag="w", bufs=2, name="t_wv")
            for dc in range(DC):
                nc.sync.dma_start(t_wv[:, dc, :], wv[:, dc, :])
            # V native: psum[k-tile, dout-half] = sum_dc memT[:,dc,ktile].T @ wv[:,dc,dh]
            for dh in range(2):
                for kt in range(H):
                    pt = ps.tile([P, SH], F32, tag="mm", bufs=2,
                                 name=f"p_v_{kt}_{dh}")
                    for dc in range(DC):
                        nc.tensor.matmul(
                            pt[:],
                            t_memT[:, dc, kt * P:(kt + 1) * P],
                            t_wv[:, dc, dh * SH:(dh + 1) * SH],
                            start=(dc == 0), stop=(dc == DC - 1))
                    nc.vector.tensor_copy(
                        t_v[:, kt, dh * SH:(dh + 1) * SH], pt[:])

            # ============ Phase B: attention per head ============
            def emit_rep(j, t_rbf, t_rrep, xparts):
                # replicate r across partitions, then scale this head's
                # attn@V psum tiles into xoutT (deferred to overlap with the
                # next head's scores so the PE never stalls on the DVE chain)
                for qh in range(2):
                    pt = ps.tile([P, SH], F32, tag="rep", bufs=1,
                                 name=f"p_rep_{j}_{qh}")
                    nc.tensor.matmul(pt[:], t_ones_r[:],
                                     t_rbf[:, qh * SH:(qh + 1) * SH],
                                     start=True, stop=True)
                    nc.vector.tensor_copy(t_rrep[:, qh * SH:(qh + 1) * SH],
                                          pt[:])
                for qh in range(2):
                    nc.vector.tensor_mul(
                        t_xoutT[:, j, qh * SH:(qh + 1) * SH], xparts[qh],
                        t_rrep[:, qh * SH:(qh + 1) * SH])

            def emit_scores(j):
                t_exp = sb.tile([P, DC, S], BF16, tag="big_rot", bufs=2,
                                name=f"t_exp_{j}")
                for i in range(DC):
                    pt = ps.tile([P, 2, SH], F32, tag="s", bufs=2,
                                 name=f"p_s_{j}_{i}")
                    for qh in range(2):
                        nc.tensor.matmul(
                            pt[:, qh, :],
                            t_kt[:, j, i * P:(i + 1) * P],
                            t_qt[:, j, qh * SH:(qh + 1) * SH],
                            start=True, stop=True)
                    t_eraw = sb.tile([P, S], BF16, tag="wqc_eraw", bufs=8,
                                     name=f"t_eraw_{j}_{i}")
                    nc.scalar.activation(
                        out=t_eraw[:],
                        in_=pt[:].rearrange("p a b -> p (a b)"),
                        func=mybir.ActivationFunctionType.Exp, scale=SCALE)
                    mask_eng = nc.gpsimd if i % 4 == 3 else nc.vector
                    mask_eng.tensor_mul(
                        t_exp[:, i, :], t_eraw[:], t_keepT[:, i, :])
                    nc.sync.dma_start(expt[j, i * P:(i + 1) * P, :],
                                      t_exp[:, i, :])
                return t_exp

            def emit_reduce(j, t_exp):
                # softmax denominators, replicated across partitions via an
                # all-ones [P, P] stationary operand: psum[m, q] = sum_k exp
                t_rrep = sb.tile([P, S], F32, tag="rrep", bufs=1,
                                 name=f"t_rrep_{j}")
                for qh in range(2):
                    pt = ps.tile([P, SH], F32, tag="sum", bufs=2,
                                 name=f"p_sum_{j}_{qh}")
                    for i in range(DC):
                        nc.tensor.matmul(
                            pt[:], t_ones_c[:],
                            t_exp[:, i, qh * SH:(qh + 1) * SH],
                            start=(i == 0), stop=(i == DC - 1))
                    nc.vector.reciprocal(t_rrep[:, qh * SH:(qh + 1) * SH],
                                         pt[:])
                nc.vector.tensor_mul(t_rrep[:], t_rrep[:], t_qmask[:])
                nc.sync.dma_start(r_out[j:j + 1, :], t_rrep[0:1, :])

                # attn @ V (transposed): x^T[dv, q] = sum_k V[k, dv] expT[k, q]
                for qh in range(2):
                    pt = ps.tile([P, SH], F32, tag="mm", bufs=2,
                                 name=f"p_x_{j}_{qh}")
                    for i in range(DC):
                        nc.tensor.matmul(
                            pt[:],
                            t_v[:, i, j * P:(j + 1) * P],
                            t_exp[:, i, qh * SH:(qh + 1) * SH],
                            start=(i == 0), stop=(i == DC - 1))
                    nc.vector.tensor_mul(
                        t_xoutT[:, j, qh * SH:(qh + 1) * SH], pt[:],
                        t_rrep[:, qh * SH:(qh + 1) * SH])

            prev = None
            for j in range(H):
                t_exp = emit_scores(j)
                if prev is not None:
                    emit_reduce(prev[0], prev[1])
                prev = (j, t_exp)
            emit_reduce(prev[0], prev[1])

            # ============ Phase C: final linear + residual + LayerNorm ============
            t_wft = sb.tile([P, DC, D], BF16, tag="w", bufs=2, name="t_wft")
            t_wfb = sb.tile([P, DC, D], BF16, tag="w", bufs=2, name="t_wfb")
            for dc in range(DC):
                nc.sync.dma_start(t_wft[:, dc, :], wf_top[:, dc, :])
            for dc in range(DC):
                nc.sync.dma_start(t_wfb[:, dc, :], wf_bot[:, dc, :])

            for t in range(H):
                t_dec = sb.tile([P, D], F32, tag="dec", bufs=2,
                                name=f"t_dec_{t}")
                nc.sync.dma_start(t_dec[:], dec_res[t * P:(t + 1) * P, :])
                t_x2 = sb.tile([P, D], F32, tag="x2", bufs=2, name=f"t_x2_{t}")
                for n in range(2):
                    pt = ps.tile([P, SH], F32, tag="mm", bufs=2,
                                 name=f"p_f_{t}_{n}")
                    for dc in range(DC):
                        nc.tensor.matmul(
                            pt[:],
                            t_decT_c[dc][:, t * P:(t + 1) * P],
                            t_wft[:, dc, n * SH:(n + 1) * SH],
                            start=(dc == 0), stop=False)
                    for j in range(H):
                        nc.tensor.matmul(
                            pt[:],
                            t_xoutT[:, j, t * P:(t + 1) * P],
                            t_wfb[:, j, n * SH:(n + 1) * SH],
                            start=False, stop=(j == H - 1))
                    nc.vector.tensor_add(t_x2[:, n * SH:(n + 1) * SH], pt[:],
                                         t_dec[:, n * SH:(n + 1) * SH])

                # LayerNorm along the free dim (D = 1024, 2 bn_stats subgroups)
                t_stats = sb.tile([P, 2, 6], F32, tag="stats", bufs=2,
                                  name=f"t_stats_{t}")
                for g in range(2):
                    nc.vector.bn_stats(out=t_stats[:, g, :],
                                       in_=t_x2[:, g * SH:(g + 1) * SH])
                t_mv = sb.tile([P, 2], F32, tag="mv", bufs=2, name=f"t_mv_{t}")
                nc.vector.bn_aggr(out=t_mv[:], in_=t_stats[:])
                t_rstd = sb.tile([P, 1], F32, tag="rstd", bufs=2,
                                 name=f"t_rstd_{t}")
                nc.scalar.activation(
                    out=t_rstd[:], in_=t_mv[:, 1:2],
                    func=mybir.ActivationFunctionType.Sqrt,
                    bias=t_eps[:], scale=1.0)
                nc.vector.reciprocal(t_rstd[:], t_rstd[:])
                nc.vector.tensor_scalar(
                    t_x2[:], t_x2[:], scalar1=t_mv[:, 0:1], scalar2=t_rstd[:],
                    op0=mybir.AluOpType.subtract, op1=mybir.AluOpType.mult)
                nc.sync.dma_start(x_out[t * P:(t + 1) * P, :], t_x2[:])

    nc.compile()
    return nc


def _stage_t(arr):
    """[S, D]-like 2D -> [P, DC, S] bf16 with dim0 = (dc, dp) transposed."""
    return np.ascontiguousarray(
        arr.T.reshape(DC, P, -1).transpose(1, 0, 2)).astype(BF)


def _stage_w(w):
    """[D, D] weight -> [P, DC, D] bf16 (contraction rows onto partitions)."""
    return np.ascontiguousarray(
        w.reshape(DC, P, -1).transpose(1, 0, 2)).astype(BF)


def kernel(memory, decoder_input, query_mask, Wk, Wv, Wq, Wf, bf, gamma, beta,
           mask):
    memory = np.asarray(memory, np.float32)
    decoder_input = np.asarray(decoder_input, np.float32)
    query_mask = np.asarray(query_mask, np.float32)
    Wk = np.asarray(Wk, np.float32)
    Wv = np.asarray(Wv, np.float32)
    Wq = np.asarray(Wq, np.float32)
    Wf = np.asarray(Wf, np.float32)
    bf = np.asarray(bf, np.float32)
    gamma = np.asarray(gamma, np.float32)
    beta = np.asarray(beta, np.float32)
    mask = np.asarray(mask)

    if "nc" not in _CACHE:
        _CACHE["nc"] = build()
    nc = _CACHE["nc"]

    shared = dict(
        wq=_stage_w(Wq), wk=_stage_w(Wk), wv=_stage_w(Wv),
        wf_top=_stage_w(Wf[:D]), wf_bot=_stage_w(Wf[D:]),
        ones_c=np.ones((P, P), BF),
    )
    in_maps = []
    for b in range(B):
        in_maps.append(dict(
            shared,
            decT=_stage_t(decoder_input[b]),
            memT=_stage_t(memory[b]),
            keepT=np.ascontiguousarray(
                (~mask[b]).T.astype(np.float32)
                .reshape(DC, P, S).transpose(1, 0, 2)).astype(BF),
            dec_res=np.ascontiguousarray(decoder_input[b] + bf[None, :],
                                         dtype=np.float32),
            qmask=np.ascontiguousarray(
                np.broadcast_to(query_mask[b][None, :], (P, S)),
                dtype=np.float32),
        ))

    res = run_bass_kernel_spmd(nc, in_maps, core_ids=list(range(B)),
                               **_CACHE.get("run_kwargs", {}))
    _CACHE["last_result"] = res

    x = np.empty((B, S, D), np.float32)
    attns = np.empty((H * B, S, S), np.float32)
    apply_gb = (not np.all(gamma == 1.0)) or (not np.all(beta == 0.0))
    for b in range(B):
        rb = res.results[b]
        if apply_gb:
            x[b] = rb["x_out"] * gamma[None, :] + beta[None, :]
        else:
            x[b] = rb["x_out"]
        e = rb["expt"].astype(np.float32)          # [H, k, q]
        r = rb["r_out"]                            # [H, q]
        for j in range(H):
            np.multiply(e[j].T, r[j][:, None], out=attns[j * B + b])
    return x, attns


if __name__ == "__main__":
    rng = np.random.default_rng(0)
    ins = dict(
        memory=rng.standard_normal((B, S, D), dtype=np.float32),
        decoder_input=rng.standard_normal((B, S, D), dtype=np.float32),
        query_mask=rng.random((B, S), dtype=np.float32),
        Wk=(rng.standard_normal((D, D), dtype=np.float32) * 0.02),
        Wv=(rng.standard_normal((D, D), dtype=np.float32) * 0.02),
        Wq=(rng.standard_normal((D, D), dtype=np.float32) * 0.02),
        Wf=(rng.standard_normal((2 * D, D), dtype=np.float32) * 0.02),
        bf=np.zeros(D, np.float32),
        gamma=np.ones(D, np.float32),
        beta=np.zeros(D, np.float32),
        mask=rng.integers(0, 2, (B, S, S)) == 1,
    )
    x, attns = kernel(**ins)
    print("ran", x.shape, attns.shape)


# revision 23
# speedup vs baseline: 1.1464x; 1.0135x over previous
"""Trainium2 Bass kernel for the nn_Attention problem.

Cross-attention transformer block: QKV projections, masked softmax
attention with a post-softmax query-mask multiply, concat + final linear,
residual, LayerNorm.  Returns (x, attns) like the reference.

Sharding: data-parallel over batch B=8 across the 8 NeuronCores — each
core computes one batch element end-to-end; no collectives.

Device-side layout trick: all attention tensors are kept "transposed"
(contraction dim on partitions) so every matmul lhsT/rhs is a natural
slice — the host pre-transposes decoder/memory/mask per core, and the
attention weights output is produced as expT [k, q] + row-normalizers r,
with the final attns = (expT.T * r) assembled on the host.
"""
import sys, os

for p in ("/opt/trn_rl_repo",):
    if p not in sys.path and os.path.isdir(p):
        sys.path.insert(0, p)

import numpy as np
import ml_dtypes

import concourse.bass as bass
import concourse.mybir as mybir
import concourse.tile as tile
from concourse import bacc
from concourse.bass_utils import run_bass_kernel_spmd

F32 = mybir.dt.float32
BF16 = mybir.dt.bfloat16
BF = ml_dtypes.bfloat16

B, S, D, H, P = 8, 1024, 1024, 8, 128
DC = D // P          # 8 contraction chunks of 128
SH = 512             # free-dim half
NEG = float(-2**32 + 1)
SCALE = float(1.0 / np.sqrt(P))  # 1/sqrt(dh), dh = 128
LN_EPS = 1e-5

_CACHE = {}


def build():
    nc = bacc.Bacc(None, target_bir_lowering=False)

    # ---- per-core inputs (bf16 matmul operands staged by host) ----
    decT = nc.dram_tensor("decT", [P, DC, S], BF16, kind="ExternalInput")
    memT = nc.dram_tensor("memT", [P, DC, S], BF16, kind="ExternalInput")
    keepT = nc.dram_tensor("keepT", [P, DC, S], BF16, kind="ExternalInput")
    dec_res = nc.dram_tensor("dec_res", [S, D], F32, kind="ExternalInput")
    qmask = nc.dram_tensor("qmask", [P, S], F32, kind="ExternalInput")
    # ---- shared weights / constants ----
    wq = nc.dram_tensor("wq", [P, DC, D], BF16, kind="ExternalInput")
    wk = nc.dram_tensor("wk", [P, DC, D], BF16, kind="ExternalInput")
    wv = nc.dram_tensor("wv", [P, DC, D], BF16, kind="ExternalInput")
    wf_top = nc.dram_tensor("wf_top", [P, DC, D], BF16, kind="ExternalInput")
    wf_bot = nc.dram_tensor("wf_bot", [P, DC, D], BF16, kind="ExternalInput")
    ones_c = nc.dram_tensor("ones_c", [P, P], BF16, kind="ExternalInput")

    # ---- outputs ----
    x_out = nc.dram_tensor("x_out", [S, D], F32, kind="ExternalOutput")
    expt = nc.dram_tensor("expt", [H, S, S], BF16, kind="ExternalOutput")
    r_out = nc.dram_tensor("r_out", [H, S], F32, kind="ExternalOutput")

    with tile.TileContext(nc) as tc:
        with (
            tc.tile_pool(name="sb", bufs=1) as sb,
            tc.tile_pool(name="ps", bufs=1, space="PSUM") as ps,
        ):
            # ---------- constants ----------
            t_ones_c = sb.tile([P, P], BF16)
            t_qmask = sb.tile([P, S], F32)
            t_eps = sb.tile([P, 1], F32)
            nc.vector.memset(t_eps[:], LN_EPS)

            # ---------- persistent big tiles ----------
            # decT/wq as per-chunk tiles so the first projection matmuls can
            # start as soon as chunk 0 lands (deps are tracked per tile)
            t_decT_c = [sb.tile([P, S], BF16, name=f"t_decT_{dc}")
                        for dc in range(DC)]
            t_keepT = sb.tile([P, DC, S], BF16)
            # (keepT DMA deferred below so early-phase DMAs get the bandwidth)
            t_qt = sb.tile([P, H, S], BF16)      # Q^T  [dh, head, q]
            t_kt = sb.tile([P, H, S], BF16)      # K^T  [dh, head, k]
            t_v = sb.tile([P, DC, D], BF16)      # V    [k_inner, k_chunk, dout]
            t_xoutT = sb.tile([P, H, S], BF16)   # attn_out^T [dh, head, q]
            for dc in range(DC):
                eng = nc.sync if dc % 2 == 0 else nc.scalar
                eng.dma_start(t_decT_c[dc][:], decT[:, dc, :])

            # ---------- PE warm-up ----------
            # ~20 throwaway matmuls keep the PE HAM activity monitor busy
            # while the first input DMAs land, so real matmuls start at the
            # full 2.4 GHz clock instead of the cold 1.2 GHz state.
            t_warm = sb.tile([P, SH], BF16)
            nc.vector.memset(t_warm[:], 0.0)
            p_warm = ps.tile([P, SH], F32, tag="mm", bufs=2, name="p_warm")
            for _ in range(20):
                nc.tensor.matmul(p_warm[:], t_warm[:, :P], t_warm[:],
                                 start=True, stop=True)

            # ============ Phase A: projections ============
            # memT shares a rotation tag with the per-head expT tiles.
            t_memT = sb.tile([P, DC, S], BF16, tag="big_rot", bufs=2,
                             name="t_memT")
            for dc in range(DC):
                nc.sync.dma_start(t_memT[:, dc, :], memT[:, dc, :])
            t_wq_c = [sb.tile([P, D], BF16, tag="wqc_eraw", bufs=8,
                                name=f"t_wq_{dc}")
                      for dc in range(DC)]
            t_wk = sb.tile([P, DC, D], BF16, tag="w", bufs=2, name="t_wk")
            for dc in range(DC):
                eng = nc.scalar if dc % 2 == 0 else nc.sync
                eng.dma_start(t_wq_c[dc][:], wq[:, dc, :])
            for dc in range(DC):
                nc.sync.dma_start(t_wk[:, dc, :], wk[:, dc, :])

            def proj(dst, w_sl, rhs_sl):
                # dst[:, m, qh] = sum_dc w[dc][:, m-tile].T @ rhs[dc][:, qh]
                for qh in range(2):
                    for m in range(H):
                        pt = ps.tile([P, SH], F32, tag="mm", bufs=2,
                                     name=f"p_mm_{m}_{qh}")
                        for dc in range(DC):
                            nc.tensor.matmul(
                                pt[:],
                                w_sl(dc)[:, m * P:(m + 1) * P],
                                rhs_sl(dc)[:, qh * SH:(qh + 1) * SH],
                                start=(dc == 0), stop=(dc == DC - 1))
                        nc.vector.tensor_copy(
                            dst[:, m, qh * SH:(qh + 1) * SH], pt[:])

            proj(t_qt, lambda dc: t_wq_c[dc][:], lambda dc: t_decT_c[dc][:])

            for dc in range(DC):
                nc.sync.dma_start(t_keepT[:, dc, :], keepT[:, dc, :])
            nc.sync.dma_start(t_ones_c[:], ones_c[:])
            nc.sync.dma_start(t_qmask[:], qmask[:])

            proj(t_kt, lambda dc: t_wk[:, dc, :], lambda dc: t_memT[:, dc, :])

            t_wv = sb.tile([P, DC, D], BF16, tag="w", bufs=2, name="t_wv")
            for dc in range(DC):
                nc.sync.dma_start(t_wv[:, dc, :], wv[:, dc, :])
            # V native: psum[k-tile, dout-half] = sum_dc memT[:,dc,ktile].T @ wv[:,dc,dh]
            for dh in range(2):
                for kt in range(H):
                    pt = ps.tile([P, SH], F32, tag="mm", bufs=2,
                                 name=f"p_v_{kt}_{dh}")
                    for dc in range(DC):
                        nc.tensor.matmul(
                            pt[:],
                            t_memT[:, dc, kt * P:(kt + 1) * P],
                            t_wv[:, dc, dh * SH:(dh + 1) * SH],
                            start=(dc == 0), stop=(dc == DC - 1))
                    nc.vector.tensor_copy(
                        t_v[:, kt, dh * SH:(dh + 1) * SH], pt[:])

            # ============ Phase B: attention per head ============
            def emit_rep(j, t_rbf, t_rrep, xparts):
                # replicate r across partitions, then scale this head's
                # attn@V psum tiles into xoutT (deferred to overlap with the
                # next head's scores so the PE never stalls on the DVE chain)
                for qh in range(2):
                    pt = ps.tile([P, SH], F32, tag="rep", bufs=1,
                                 name=f"p_rep_{j}_{qh}")
                    nc.tensor.matmul(pt[:], t_ones_r[:],
                                     t_rbf[:, qh * SH:(qh + 1) * SH],
                                     start=True, stop=True)
                    nc.vector.tensor_copy(t_rrep[:, qh * SH:(qh + 1) * SH],
                                          pt[:])
                for qh in range(2):
                    nc.vector.tensor_mul(
                        t_xoutT[:, j, qh * SH:(qh + 1) * SH], xparts[qh],
                        t_rrep[:, qh * SH:(qh + 1) * SH])

            def emit_scores(j):
                t_exp = sb.tile([P, DC, S], BF16, tag="big_rot", bufs=2,
                                name=f"t_exp_{j}")
                for i in range(DC):
                    pt = ps.tile([P, 2, SH], F32, tag="s", bufs=2,
                                 name=f"p_s_{j}_{i}")
                    for qh in range(2):
                        nc.tensor.matmul(
                            pt[:, qh, :],
                            t_kt[:, j, i * P:(i + 1) * P],
                            t_qt[:, j, qh * SH:(qh + 1) * SH],
                            start=True, stop=True)
                    t_eraw = sb.tile([P, S], BF16, tag="wqc_eraw", bufs=8,
                                     name=f"t_eraw_{j}_{i}")
                    nc.scalar.activation(
                        out=t_eraw[:],
                        in_=pt[:].rearrange("p a b -> p (a b)"),
                        func=mybir.ActivationFunctionType.Exp, scale=SCALE)
                    mask_eng = nc.gpsimd if i % 4 == 3 else nc.vector
                    mask_eng.tensor_mul(
                        t_exp[:, i, :], t_eraw[:], t_keepT[:, i, :])
                    nc.sync.dma_start(expt[j, i * P:(i + 1) * P, :],
                                      t_exp[:, i, :])
                return t_exp

            def emit_reduce(j, t_exp):
                # softmax denominators, replicated across partitions via an
                # all-ones [P, P] stationary operand: psum[m, q] = sum_k exp
                t_rrep = sb.tile([P, S], F32, tag="rrep", bufs=1,
                                 name=f"t_rrep_{j}")
                for qh in range(2):
                    pt = ps.tile([P, SH], F32, tag="sum", bufs=2,
                                 name=f"p_sum_{j}_{qh}")
                    for i in range(DC):
                        nc.tensor.matmul(
                            pt[:], t_ones_c[:],
                            t_exp[:, i, qh * SH:(qh + 1) * SH],
                            start=(i == 0), stop=(i == DC - 1))
                    # 1/x as exp(-log(x)) — two fast ACT table ops; the DVE
                    # reciprocal is ~6.4 cycles/element and would pace phase B
                    t_lg = sb.tile([P, SH], F32, tag="lg", bufs=2,
                                   name=f"t_lg_{j}_{qh}")
                    nc.scalar.activation(
                        out=t_lg[:], in_=pt[:],
                        func=mybir.ActivationFunctionType.Ln, scale=1.0)
                    nc.scalar.activation(
                        out=t_rrep[:, qh * SH:(qh + 1) * SH], in_=t_lg[:],
                        func=mybir.ActivationFunctionType.Exp, scale=-1.0)
                nc.vector.tensor_mul(t_rrep[:], t_rrep[:], t_qmask[:])
                nc.sync.dma_start(r_out[j:j + 1, :], t_rrep[0:1, :])

                # attn @ V (transposed): x^T[dv, q] = sum_k V[k, dv] expT[k, q]
                for qh in range(2):
                    pt = ps.tile([P, SH], F32, tag="mm", bufs=2,
                                 name=f"p_x_{j}_{qh}")
                    for i in range(DC):
                        nc.tensor.matmul(
                            pt[:],
                            t_v[:, i, j * P:(j + 1) * P],
                            t_exp[:, i, qh * SH:(qh + 1) * SH],
                            start=(i == 0), stop=(i == DC - 1))
                    nc.vector.tensor_mul(
                        t_xoutT[:, j, qh * SH:(qh + 1) * SH], pt[:],
                        t_rrep[:, qh * SH:(qh + 1) * SH])

            prev = None
            for j in range(H):
                t_exp = emit_scores(j)
                if prev is not None:
                    emit_reduce(prev[0], prev[1])
                prev = (j, t_exp)
            emit_reduce(prev[0], prev[1])

            # ============ Phase C: final linear + residual + LayerNorm ============
            t_wft = sb.tile([P, DC, D], BF16, tag="w", bufs=2, name="t_wft")
            t_wfb = sb.tile([P, DC, D], BF16, tag="w", bufs=2, name="t_wfb")
            for dc in range(DC):
                nc.sync.dma_start(t_wft[:, dc, :], wf_top[:, dc, :])
            for dc in range(DC):
                nc.sync.dma_start(t_wfb[:, dc, :], wf_bot[:, dc, :])

            for t in range(H):
                t_dec = sb.tile([P, D], F32, tag="dec", bufs=2,
                                name=f"t_dec_{t}")
                nc.sync.dma_start(t_dec[:], dec_res[t * P:(t + 1) * P, :])
                t_x2 = sb.tile([P, D], F32, tag="x2", bufs=2, name=f"t_x2_{t}")
                for n in range(2):
                    pt = ps.tile([P, SH], F32, tag="mm", bufs=2,
                                 name=f"p_f_{t}_{n}")
                    for dc in range(DC):
                        nc.tensor.matmul(
                            pt[:],
                            t_decT_c[dc][:, t * P:(t + 1) * P],
                            t_wft[:, dc, n * SH:(n + 1) * SH],
                            start=(dc == 0), stop=False)
                    for j in range(H):
                        nc.tensor.matmul(
                            pt[:],
                            t_xoutT[:, j, t * P:(t + 1) * P],
                            t_wfb[:, j, n * SH:(n + 1) * SH],
                            start=False, stop=(j == H - 1))
                    nc.vector.tensor_add(t_x2[:, n * SH:(n + 1) * SH], pt[:],
                                         t_dec[:, n * SH:(n + 1) * SH])

                # LayerNorm along the free dim (D = 1024, 2 bn_stats subgroups)
                t_stats = sb.tile([P, 2, 6], F32, tag="stats", bufs=2,
                                  name=f"t_stats_{t}")
                for g in range(2):
                    nc.vector.bn_stats(out=t_stats[:, g, :],
                                       in_=t_x2[:, g * SH:(g + 1) * SH])
                t_mv = sb.tile([P, 2], F32, tag="mv", bufs=2, name=f"t_mv_{t}")
                nc.vector.bn_aggr(out=t_mv[:], in_=t_stats[:])
                t_rstd = sb.tile([P, 1], F32, tag="rstd", bufs=2,
                                 name=f"t_rstd_{t}")
                nc.scalar.activation(
                    out=t_rstd[:], in_=t_mv[:, 1:2],
                    func=mybir.ActivationFunctionType.Sqrt,
                    bias=t_eps[:], scale=1.0)
                nc.vector.reciprocal(t_rstd[:], t_rstd[:])
                nc.vector.tensor_scalar(
                    t_x2[:], t_x2[:], scalar1=t_mv[:, 0:1], scalar2=t_rstd[:],
                    op0=mybir.AluOpType.subtract, op1=mybir.AluOpType.mult)
                nc.sync.dma_start(x_out[t * P:(t + 1) * P, :], t_x2[:])

    nc.compile()
    return nc


def _stage_t(arr):
    """[S, D]-like 2D -> [P, DC, S] bf16 with dim0 = (dc, dp) transposed."""
    return np.ascontiguousarray(
        arr.T.reshape(DC, P, -1).transpose(1, 0, 2)).astype(BF)


def _stage_w(w):
    """[D, D] weight -> [P, DC, D] bf16 (contraction rows onto partitions)."""
    return np.ascontiguousarray(
        w.reshape(DC, P, -1).transpose(1, 0, 2)).astype(BF)


def kernel(memory, decoder_input, query_mask, Wk, Wv, Wq, Wf, bf, gamma, beta,
           mask):
    memory = np.asarray(memory, np.float32)
    decoder_input = np.asarray(decoder_input, np.float32)
    query_mask = np.asarray(query_mask, np.float32)
    Wk = np.asarray(Wk, np.float32)
    Wv = np.asarray(Wv, np.float32)
    Wq = np.asarray(Wq, np.float32)
    Wf = np.asarray(Wf, np.float32)
    bf = np.asarray(bf, np.float32)
    gamma = np.asarray(gamma, np.float32)
    beta = np.asarray(beta, np.float32)
    mask = np.asarray(mask)

    if "nc" not in _CACHE:
        _CACHE["nc"] = build()
    nc = _CACHE["nc"]

    shared = dict(
        wq=_stage_w(Wq), wk=_stage_w(Wk), wv=_stage_w(Wv),
        wf_top=_stage_w(Wf[:D]), wf_bot=_stage_w(Wf[D:]),
        ones_c=np.ones((P, P), BF),
    )
    in_maps = []
    for b in range(B):
        in_maps.append(dict(
            shared,
            decT=_stage_t(decoder_input[b]),
            memT=_stage_t(memory[b]),
            keepT=np.ascontiguousarray(
                (~mask[b]).T.astype(np.float32)
                .reshape(DC, P, S).transpose(1, 0, 2)).astype(BF),
            dec_res=np.ascontiguousarray(decoder_input[b] + bf[None, :],
                                         dtype=np.float32),
            qmask=np.ascontiguousarray(
                np.broadcast_to(query_mask[b][None, :], (P, S)),
                dtype=np.float32),
        ))

    res = run_bass_kernel_spmd(nc, in_maps, core_ids=list(range(B)),
                               **_CACHE.get("run_kwargs", {}))
    _CACHE["last_result"] = res

    x = np.empty((B, S, D), np.float32)
    attns = np.empty((H * B, S, S), np.float32)
    apply_gb = (not np.all(gamma == 1.0)) or (not np.all(beta == 0.0))
    for b in range(B):
        rb = res.results[b]
        if apply_gb:
            x[b] = rb["x_out"] * gamma[None, :] + beta[None, :]
        else:
            x[b] = rb["x_out"]
        e = rb["expt"].astype(np.float32)          # [H, k, q]
        r = rb["r_out"]                            # [H, q]
        for j in range(H):
            np.multiply(e[j].T, r[j][:, None], out=attns[j * B + b])
    return x, attns


if __name__ == "__main__":
    rng = np.random.default_rng(0)
    ins = dict(
        memory=rng.standard_normal((B, S, D), dtype=np.float32),
        decoder_input=rng.standard_normal((B, S, D), dtype=np.float32),
        query_mask=rng.random((B, S), dtype=np.float32),
        Wk=(rng.standard_normal((D, D), dtype=np.float32) * 0.02),
        Wv=(rng.standard_normal((D, D), dtype=np.float32) * 0.02),
        Wq=(rng.standard_normal((D, D), dtype=np.float32) * 0.02),
        Wf=(rng.standard_normal((2 * D, D), dtype=np.float32) * 0.02),
        bf=np.zeros(D, np.float32),
        gamma=np.ones(D, np.float32),
        beta=np.zeros(D, np.float32),
        mask=rng.integers(0, 2, (B, S, S)) == 1,
    )
    x, attns = kernel(**ins)
    print("ran", x.shape, attns.shape)


# revision 24
# speedup vs baseline: 1.2470x; 1.0877x over previous
"""Trainium2 Bass kernel for the nn_Attention problem.

Cross-attention transformer block: QKV projections, masked softmax
attention with a post-softmax query-mask multiply, concat + final linear,
residual, LayerNorm.  Returns (x, attns) like the reference.

Sharding: data-parallel over batch B=8 across the 8 NeuronCores — each
core computes one batch element end-to-end; no collectives.

Device-side layout trick: all attention tensors are kept "transposed"
(contraction dim on partitions) so every matmul lhsT/rhs is a natural
slice — the host pre-transposes decoder/memory/mask per core, and the
attention weights output is produced as expT [k, q] + row-normalizers r,
with the final attns = (expT.T * r) assembled on the host.
"""
import sys, os

for p in ("/opt/trn_rl_repo",):
    if p not in sys.path and os.path.isdir(p):
        sys.path.insert(0, p)

import numpy as np
import ml_dtypes

import concourse.bass as bass
import concourse.mybir as mybir
import concourse.tile as tile
from concourse import bacc
from concourse.bass_utils import run_bass_kernel_spmd

F32 = mybir.dt.float32
BF16 = mybir.dt.bfloat16
BF = ml_dtypes.bfloat16

B, S, D, H, P = 8, 1024, 1024, 8, 128
DC = D // P          # 8 contraction chunks of 128
SH = 512             # free-dim half
NEG = float(-2**32 + 1)
SCALE = float(1.0 / np.sqrt(P))  # 1/sqrt(dh), dh = 128
LN_EPS = 1e-5

_CACHE = {}


def _patch_act_tables():
    """Steer the ACT table-set chooser to the combined ln+exp set so the
    per-head 1/x = exp(-ln(x)) pair doesn't ping-pong table loads
    (~1.3 us per swap, 2 swaps per head otherwise)."""
    import concourse.hw_specs as hw_specs
    orig = hw_specs.get_activation_tables
    EXP = mybir.ActivationFunctionType.Exp
    LN = mybir.ActivationFunctionType.Ln

    def patched(module_arch):
        tables = orig(module_arch)
        for name, fns in tables.items():
            if name != "natural_log_exp_and_others":
                fns.discard(EXP)
                fns.discard(LN)
        return tables

    bacc.get_activation_tables = patched


def build():
    _patch_act_tables()
    nc = bacc.Bacc(None, target_bir_lowering=False)

    # ---- per-core inputs (bf16 matmul operands staged by host) ----
    decT = nc.dram_tensor("decT", [P, DC, S], BF16, kind="ExternalInput")
    memT = nc.dram_tensor("memT", [P, DC, S], BF16, kind="ExternalInput")
    keepT = nc.dram_tensor("keepT", [P, DC, S], BF16, kind="ExternalInput")
    dec_res = nc.dram_tensor("dec_res", [S, D], F32, kind="ExternalInput")
    qmask = nc.dram_tensor("qmask", [P, S], F32, kind="ExternalInput")
    # ---- shared weights / constants ----
    wq = nc.dram_tensor("wq", [P, DC, D], BF16, kind="ExternalInput")
    wk = nc.dram_tensor("wk", [P, DC, D], BF16, kind="ExternalInput")
    wv = nc.dram_tensor("wv", [P, DC, D], BF16, kind="ExternalInput")
    wf_top = nc.dram_tensor("wf_top", [P, DC, D], BF16, kind="ExternalInput")
    wf_bot = nc.dram_tensor("wf_bot", [P, DC, D], BF16, kind="ExternalInput")
    ones_c = nc.dram_tensor("ones_c", [P, P], BF16, kind="ExternalInput")

    # ---- outputs ----
    x_out = nc.dram_tensor("x_out", [S, D], F32, kind="ExternalOutput")
    expt = nc.dram_tensor("expt", [H, S, S], BF16, kind="ExternalOutput")
    r_out = nc.dram_tensor("r_out", [H, S], F32, kind="ExternalOutput")

    with tile.TileContext(nc) as tc:
        with (
            tc.tile_pool(name="sb", bufs=1) as sb,
            tc.tile_pool(name="ps", bufs=1, space="PSUM") as ps,
        ):
            # ---------- constants ----------
            t_ones_c = sb.tile([P, P], BF16)
            t_qmask = sb.tile([P, S], F32)
            t_eps = sb.tile([P, 1], F32)
            nc.vector.memset(t_eps[:], LN_EPS)

            # ---------- persistent big tiles ----------
            # decT/wq as per-chunk tiles so the first projection matmuls can
            # start as soon as chunk 0 lands (deps are tracked per tile)
            t_decT_c = [sb.tile([P, S], BF16, name=f"t_decT_{dc}")
                        for dc in range(DC)]
            t_keepT = sb.tile([P, DC, S], BF16)
            # (keepT DMA deferred below so early-phase DMAs get the bandwidth)
            t_qt = sb.tile([P, H, S], BF16)      # Q^T  [dh, head, q]
            t_kt = sb.tile([P, H, S], BF16)      # K^T  [dh, head, k]
            t_v = sb.tile([P, DC, D], BF16)      # V    [k_inner, k_chunk, dout]
            t_xoutT = sb.tile([P, H, S], BF16)   # attn_out^T [dh, head, q]
            for dc in range(DC):
                eng = nc.sync if dc % 2 == 0 else nc.scalar
                eng.dma_start(t_decT_c[dc][:], decT[:, dc, :])

            # ---------- PE warm-up ----------
            # ~20 throwaway matmuls keep the PE HAM activity monitor busy
            # while the first input DMAs land, so real matmuls start at the
            # full 2.4 GHz clock instead of the cold 1.2 GHz state.
            t_warm = sb.tile([P, SH], BF16)
            nc.vector.memset(t_warm[:], 0.0)
            p_warm = ps.tile([P, SH], F32, tag="mm", bufs=2, name="p_warm")
            for _ in range(20):
                nc.tensor.matmul(p_warm[:], t_warm[:, :P], t_warm[:],
                                 start=True, stop=True)

            # ============ Phase A: projections ============
            # memT shares a rotation tag with the per-head expT tiles.
            t_memT = sb.tile([P, DC, S], BF16, tag="big_rot", bufs=2,
                             name="t_memT")
            for dc in range(DC):
                nc.sync.dma_start(t_memT[:, dc, :], memT[:, dc, :])
            t_wq_c = [sb.tile([P, D], BF16, tag="wqc_eraw", bufs=8,
                                name=f"t_wq_{dc}")
                      for dc in range(DC)]
            t_wk = sb.tile([P, DC, D], BF16, tag="w", bufs=2, name="t_wk")
            for dc in range(DC):
                eng = nc.scalar if dc % 2 == 0 else nc.sync
                eng.dma_start(t_wq_c[dc][:], wq[:, dc, :])
            for dc in range(DC):
                nc.sync.dma_start(t_wk[:, dc, :], wk[:, dc, :])

            def proj(dst, w_sl, rhs_sl):
                # dst[:, m, qh] = sum_dc w[dc][:, m-tile].T @ rhs[dc][:, qh]
                for qh in range(2):
                    for m in range(H):
                        pt = ps.tile([P, SH], F32, tag="mm", bufs=2,
                                     name=f"p_mm_{m}_{qh}")
                        for dc in range(DC):
                            nc.tensor.matmul(
                                pt[:],
                                w_sl(dc)[:, m * P:(m + 1) * P],
                                rhs_sl(dc)[:, qh * SH:(qh + 1) * SH],
                                start=(dc == 0), stop=(dc == DC - 1))
                        nc.vector.tensor_copy(
                            dst[:, m, qh * SH:(qh + 1) * SH], pt[:])

            proj(t_qt, lambda dc: t_wq_c[dc][:], lambda dc: t_decT_c[dc][:])

            for dc in range(DC):
                nc.sync.dma_start(t_keepT[:, dc, :], keepT[:, dc, :])
            nc.sync.dma_start(t_ones_c[:], ones_c[:])
            nc.sync.dma_start(t_qmask[:], qmask[:])

            proj(t_kt, lambda dc: t_wk[:, dc, :], lambda dc: t_memT[:, dc, :])

            t_wv = sb.tile([P, DC, D], BF16, tag="w", bufs=2, name="t_wv")
            for dc in range(DC):
                nc.sync.dma_start(t_wv[:, dc, :], wv[:, dc, :])
            # V native: psum[k-tile, dout-half] = sum_dc memT[:,dc,ktile].T @ wv[:,dc,dh]
            for dh in range(2):
                for kt in range(H):
                    pt = ps.tile([P, SH], F32, tag="mm", bufs=2,
                                 name=f"p_v_{kt}_{dh}")
                    for dc in range(DC):
                        nc.tensor.matmul(
                            pt[:],
                            t_memT[:, dc, kt * P:(kt + 1) * P],
                            t_wv[:, dc, dh * SH:(dh + 1) * SH],
                            start=(dc == 0), stop=(dc == DC - 1))
                    nc.vector.tensor_copy(
                        t_v[:, kt, dh * SH:(dh + 1) * SH], pt[:])

            # ============ Phase B: attention per head ============
            def emit_rep(j, t_rbf, t_rrep, xparts):
                # replicate r across partitions, then scale this head's
                # attn@V psum tiles into xoutT (deferred to overlap with the
                # next head's scores so the PE never stalls on the DVE chain)
                for qh in range(2):
                    pt = ps.tile([P, SH], F32, tag="rep", bufs=1,
                                 name=f"p_rep_{j}_{qh}")
                    nc.tensor.matmul(pt[:], t_ones_r[:],
                                     t_rbf[:, qh * SH:(qh + 1) * SH],
                                     start=True, stop=True)
                    nc.vector.tensor_copy(t_rrep[:, qh * SH:(qh + 1) * SH],
                                          pt[:])
                for qh in range(2):
                    nc.vector.tensor_mul(
                        t_xoutT[:, j, qh * SH:(qh + 1) * SH], xparts[qh],
                        t_rrep[:, qh * SH:(qh + 1) * SH])

            def emit_scores(j):
                t_exp = sb.tile([P, DC, S], BF16, tag="big_rot", bufs=2,
                                name=f"t_exp_{j}")
                for i in range(DC):
                    pt = ps.tile([P, 2, SH], F32, tag="s", bufs=2,
                                 name=f"p_s_{j}_{i}")
                    for qh in range(2):
                        nc.tensor.matmul(
                            pt[:, qh, :],
                            t_kt[:, j, i * P:(i + 1) * P],
                            t_qt[:, j, qh * SH:(qh + 1) * SH],
                            start=True, stop=True)
                    t_eraw = sb.tile([P, S], BF16, tag="wqc_eraw", bufs=8,
                                     name=f"t_eraw_{j}_{i}")
                    nc.scalar.activation(
                        out=t_eraw[:],
                        in_=pt[:].rearrange("p a b -> p (a b)"),
                        func=mybir.ActivationFunctionType.Exp, scale=SCALE)
                    mask_eng = nc.gpsimd if i % 4 == 3 else nc.vector
                    mask_eng.tensor_mul(
                        t_exp[:, i, :], t_eraw[:], t_keepT[:, i, :])
                    nc.sync.dma_start(expt[j, i * P:(i + 1) * P, :],
                                      t_exp[:, i, :])
                return t_exp

            def emit_reduce(j, t_exp):
                # softmax denominators, replicated across partitions via an
                # all-ones [P, P] stationary operand: psum[m, q] = sum_k exp
                t_rrep = sb.tile([P, S], F32, tag="rrep", bufs=1,
                                 name=f"t_rrep_{j}")
                for qh in range(2):
                    pt = ps.tile([P, SH], F32, tag="sum", bufs=2,
                                 name=f"p_sum_{j}_{qh}")
                    for i in range(DC):
                        nc.tensor.matmul(
                            pt[:], t_ones_c[:],
                            t_exp[:, i, qh * SH:(qh + 1) * SH],
                            start=(i == 0), stop=(i == DC - 1))
                    # 1/x as exp(-log(x)) — two fast ACT table ops; the DVE
                    # reciprocal is ~6.4 cycles/element and would pace phase B
                    t_lg = sb.tile([P, SH], F32, tag="lg", bufs=2,
                                   name=f"t_lg_{j}_{qh}")
                    nc.scalar.activation(
                        out=t_lg[:], in_=pt[:],
                        func=mybir.ActivationFunctionType.Ln, scale=1.0)
                    nc.scalar.activation(
                        out=t_rrep[:, qh * SH:(qh + 1) * SH], in_=t_lg[:],
                        func=mybir.ActivationFunctionType.Exp, scale=-1.0)
                nc.vector.tensor_mul(t_rrep[:], t_rrep[:], t_qmask[:])
                nc.sync.dma_start(r_out[j:j + 1, :], t_rrep[0:1, :])

                # attn @ V (transposed): x^T[dv, q] = sum_k V[k, dv] expT[k, q]
                for qh in range(2):
                    pt = ps.tile([P, SH], F32, tag="mm", bufs=2,
                                 name=f"p_x_{j}_{qh}")
                    for i in range(DC):
                        nc.tensor.matmul(
                            pt[:],
                            t_v[:, i, j * P:(j + 1) * P],
                            t_exp[:, i, qh * SH:(qh + 1) * SH],
                            start=(i == 0), stop=(i == DC - 1))
                    nc.vector.tensor_mul(
                        t_xoutT[:, j, qh * SH:(qh + 1) * SH], pt[:],
                        t_rrep[:, qh * SH:(qh + 1) * SH])

            prev = None
            for j in range(H):
                t_exp = emit_scores(j)
                if prev is not None:
                    emit_reduce(prev[0], prev[1])
                prev = (j, t_exp)
            emit_reduce(prev[0], prev[1])

            # ============ Phase C: final linear + residual + LayerNorm ============
            t_wft = sb.tile([P, DC, D], BF16, tag="w", bufs=2, name="t_wft")
            t_wfb = sb.tile([P, DC, D], BF16, tag="w", bufs=2, name="t_wfb")
            for dc in range(DC):
                nc.sync.dma_start(t_wft[:, dc, :], wf_top[:, dc, :])
            for dc in range(DC):
                nc.sync.dma_start(t_wfb[:, dc, :], wf_bot[:, dc, :])

            for t in range(H):
                t_dec = sb.tile([P, D], F32, tag="dec", bufs=2,
                                name=f"t_dec_{t}")
                nc.sync.dma_start(t_dec[:], dec_res[t * P:(t + 1) * P, :])
                t_x2 = sb.tile([P, D], F32, tag="x2", bufs=2, name=f"t_x2_{t}")
                for n in range(2):
                    pt = ps.tile([P, SH], F32, tag="mm", bufs=2,
                                 name=f"p_f_{t}_{n}")
                    for dc in range(DC):
                        nc.tensor.matmul(
                            pt[:],
                            t_decT_c[dc][:, t * P:(t + 1) * P],
                            t_wft[:, dc, n * SH:(n + 1) * SH],
                            start=(dc == 0), stop=False)
                    for j in range(H):
                        nc.tensor.matmul(
                            pt[:],
                            t_xoutT[:, j, t * P:(t + 1) * P],
                            t_wfb[:, j, n * SH:(n + 1) * SH],
                            start=False, stop=(j == H - 1))
                    nc.vector.tensor_add(t_x2[:, n * SH:(n + 1) * SH], pt[:],
                                         t_dec[:, n * SH:(n + 1) * SH])

                # LayerNorm along the free dim (D = 1024, 2 bn_stats subgroups)
                t_stats = sb.tile([P, 2, 6], F32, tag="stats", bufs=2,
                                  name=f"t_stats_{t}")
                for g in range(2):
                    nc.vector.bn_stats(out=t_stats[:, g, :],
                                       in_=t_x2[:, g * SH:(g + 1) * SH])
                t_mv = sb.tile([P, 2], F32, tag="mv", bufs=2, name=f"t_mv_{t}")
                nc.vector.bn_aggr(out=t_mv[:], in_=t_stats[:])
                t_rstd = sb.tile([P, 1], F32, tag="rstd", bufs=2,
                                 name=f"t_rstd_{t}")
                nc.scalar.activation(
                    out=t_rstd[:], in_=t_mv[:, 1:2],
                    func=mybir.ActivationFunctionType.Sqrt,
                    bias=t_eps[:], scale=1.0)
                nc.vector.reciprocal(t_rstd[:], t_rstd[:])
                nc.vector.tensor_scalar(
                    t_x2[:], t_x2[:], scalar1=t_mv[:, 0:1], scalar2=t_rstd[:],
                    op0=mybir.AluOpType.subtract, op1=mybir.AluOpType.mult)
                nc.sync.dma_start(x_out[t * P:(t + 1) * P, :], t_x2[:])

    nc.compile()
    return nc


def _stage_t(arr):
    """[S, D]-like 2D -> [P, DC, S] bf16 with dim0 = (dc, dp) transposed."""
    return np.ascontiguousarray(
        arr.T.reshape(DC, P, -1).transpose(1, 0, 2)).astype(BF)


def _stage_w(w):
    """[D, D] weight -> [P, DC, D] bf16 (contraction rows onto partitions)."""
    return np.ascontiguousarray(
        w.reshape(DC, P, -1).transpose(1, 0, 2)).astype(BF)


def kernel(memory, decoder_input, query_mask, Wk, Wv, Wq, Wf, bf, gamma, beta,
           mask):
    memory = np.asarray(memory, np.float32)
    decoder_input = np.asarray(decoder_input, np.float32)
    query_mask = np.asarray(query_mask, np.float32)
    Wk = np.asarray(Wk, np.float32)
    Wv = np.asarray(Wv, np.float32)
    Wq = np.asarray(Wq, np.float32)
    Wf = np.asarray(Wf, np.float32)
    bf = np.asarray(bf, np.float32)
    gamma = np.asarray(gamma, np.float32)
    beta = np.asarray(beta, np.float32)
    mask = np.asarray(mask)

    if "nc" not in _CACHE:
        _CACHE["nc"] = build()
    nc = _CACHE["nc"]

    shared = dict(
        wq=_stage_w(Wq), wk=_stage_w(Wk), wv=_stage_w(Wv),
        wf_top=_stage_w(Wf[:D]), wf_bot=_stage_w(Wf[D:]),
        ones_c=np.ones((P, P), BF),
    )
    in_maps = []
    for b in range(B):
        in_maps.append(dict(
            shared,
            decT=_stage_t(decoder_input[b]),
            memT=_stage_t(memory[b]),
            keepT=np.ascontiguousarray(
                (~mask[b]).T.astype(np.float32)
                .reshape(DC, P, S).transpose(1, 0, 2)).astype(BF),
            dec_res=np.ascontiguousarray(decoder_input[b] + bf[None, :],
                                         dtype=np.float32),
            qmask=np.ascontiguousarray(
                np.broadcast_to(query_mask[b][None, :], (P, S)),
                dtype=np.float32),
        ))

    res = run_bass_kernel_spmd(nc, in_maps, core_ids=list(range(B)),
                               **_CACHE.get("run_kwargs", {}))
    _CACHE["last_result"] = res

    x = np.empty((B, S, D), np.float32)
    attns = np.empty((H * B, S, S), np.float32)
    apply_gb = (not np.all(gamma == 1.0)) or (not np.all(beta == 0.0))
    for b in range(B):
        rb = res.results[b]
        if apply_gb:
            x[b] = rb["x_out"] * gamma[None, :] + beta[None, :]
        else:
            x[b] = rb["x_out"]
        e = rb["expt"].astype(np.float32)          # [H, k, q]
        r = rb["r_out"]                            # [H, q]
        for j in range(H):
            np.multiply(e[j].T, r[j][:, None], out=attns[j * B + b])
    return x, attns


if __name__ == "__main__":
    rng = np.random.default_rng(0)
    ins = dict(
        memory=rng.standard_normal((B, S, D), dtype=np.float32),
        decoder_input=rng.standard_normal((B, S, D), dtype=np.float32),
        query_mask=rng.random((B, S), dtype=np.float32),
        Wk=(rng.standard_normal((D, D), dtype=np.float32) * 0.02),
        Wv=(rng.standard_normal((D, D), dtype=np.float32) * 0.02),
        Wq=(rng.standard_normal((D, D), dtype=np.float32) * 0.02),
        Wf=(rng.standard_normal((2 * D, D), dtype=np.float32) * 0.02),
        bf=np.zeros(D, np.float32),
        gamma=np.ones(D, np.float32),
        beta=np.zeros(D, np.float32),
        mask=rng.integers(0, 2, (B, S, S)) == 1,
    )
    x, attns = kernel(**ins)
    print("ran", x.shape, attns.shape)


# revision 25
# speedup vs baseline: 1.2716x; 1.0197x over previous
"""Trainium2 Bass kernel for the nn_Attention problem.

Cross-attention transformer block: QKV projections, masked softmax
attention with a post-softmax query-mask multiply, concat + final linear,
residual, LayerNorm.  Returns (x, attns) like the reference.

Sharding: data-parallel over batch B=8 across the 8 NeuronCores — each
core computes one batch element end-to-end; no collectives.

Device-side layout trick: all attention tensors are kept "transposed"
(contraction dim on partitions) so every matmul lhsT/rhs is a natural
slice — the host pre-transposes decoder/memory/mask per core, and the
attention weights output is produced as expT [k, q] + row-normalizers r,
with the final attns = (expT.T * r) assembled on the host.
"""
import sys, os

for p in ("/opt/trn_rl_repo",):
    if p not in sys.path and os.path.isdir(p):
        sys.path.insert(0, p)

import numpy as np
import ml_dtypes

import concourse.bass as bass
import concourse.mybir as mybir
import concourse.tile as tile
from concourse import bacc
from concourse.bass_utils import run_bass_kernel_spmd

F32 = mybir.dt.float32
BF16 = mybir.dt.bfloat16
BF = ml_dtypes.bfloat16

B, S, D, H, P = 8, 1024, 1024, 8, 128
DC = D // P          # 8 contraction chunks of 128
SH = 512             # free-dim half
NEG = float(-2**32 + 1)
SCALE = float(1.0 / np.sqrt(P))  # 1/sqrt(dh), dh = 128
LN_EPS = 1e-5

_CACHE = {}


def _patch_act_tables():
    """Steer the ACT table-set chooser to the combined ln+exp set so the
    per-head 1/x = exp(-ln(x)) pair doesn't ping-pong table loads
    (~1.3 us per swap, 2 swaps per head otherwise)."""
    import concourse.hw_specs as hw_specs
    orig = hw_specs.get_activation_tables
    EXP = mybir.ActivationFunctionType.Exp
    LN = mybir.ActivationFunctionType.Ln

    def patched(module_arch):
        tables = orig(module_arch)
        for name, fns in tables.items():
            if name != "natural_log_exp_and_others":
                fns.discard(EXP)
                fns.discard(LN)
        return tables

    bacc.get_activation_tables = patched


def build():
    _patch_act_tables()
    nc = bacc.Bacc(None, target_bir_lowering=False)

    # ---- per-core inputs (bf16 matmul operands staged by host) ----
    decT = nc.dram_tensor("decT", [P, DC, S], BF16, kind="ExternalInput")
    memT = nc.dram_tensor("memT", [P, DC, S], BF16, kind="ExternalInput")
    keepT = nc.dram_tensor("keepT", [P, DC, S], BF16, kind="ExternalInput")
    dec_res = nc.dram_tensor("dec_res", [S, D], F32, kind="ExternalInput")
    qmask = nc.dram_tensor("qmask", [P, S], F32, kind="ExternalInput")
    # ---- shared weights / constants ----
    wq = nc.dram_tensor("wq", [P, DC, D], BF16, kind="ExternalInput")
    wk = nc.dram_tensor("wk", [P, DC, D], BF16, kind="ExternalInput")
    wv = nc.dram_tensor("wv", [P, DC, D], BF16, kind="ExternalInput")
    wf_top = nc.dram_tensor("wf_top", [P, DC, D], BF16, kind="ExternalInput")
    wf_bot = nc.dram_tensor("wf_bot", [P, DC, D], BF16, kind="ExternalInput")
    ones_c = nc.dram_tensor("ones_c", [P, P], BF16, kind="ExternalInput")

    # ---- outputs ----
    x_out = nc.dram_tensor("x_out", [S, D], F32, kind="ExternalOutput")
    expt = nc.dram_tensor("expt", [H, S, S], BF16, kind="ExternalOutput")
    r_out = nc.dram_tensor("r_out", [H, S], F32, kind="ExternalOutput")

    with tile.TileContext(nc) as tc:
        with (
            tc.tile_pool(name="sb", bufs=1) as sb,
            tc.tile_pool(name="ps", bufs=1, space="PSUM") as ps,
        ):
            # ---------- constants ----------
            t_ones_c = sb.tile([P, P], BF16)
            t_qmask = sb.tile([P, S], F32)
            t_eps = sb.tile([P, 1], F32)
            nc.vector.memset(t_eps[:], LN_EPS)

            # ---------- persistent big tiles ----------
            # decT/wq as per-chunk tiles so the first projection matmuls can
            # start as soon as chunk 0 lands (deps are tracked per tile)
            t_decT_c = [sb.tile([P, S], BF16, name=f"t_decT_{dc}")
                        for dc in range(DC)]
            t_keepT = sb.tile([P, DC, S], BF16)
            # (keepT DMA deferred below so early-phase DMAs get the bandwidth)
            t_qt = sb.tile([P, H, S], BF16)      # Q^T  [dh, head, q]
            t_kt = sb.tile([P, H, S], BF16)      # K^T  [dh, head, k]
            t_v = sb.tile([P, DC, D], BF16)      # V    [k_inner, k_chunk, dout]
            t_xoutT = sb.tile([P, H, S], BF16)   # attn_out^T [dh, head, q]


            # ---------- PE warm-up ----------
            # ~20 throwaway matmuls keep the PE HAM activity monitor busy
            # while the first input DMAs land, so real matmuls start at the
            # full 2.4 GHz clock instead of the cold 1.2 GHz state.
            t_warm = sb.tile([P, SH], BF16)
            nc.vector.memset(t_warm[:], 0.0)
            p_warm = ps.tile([P, SH], F32, tag="mm", bufs=2, name="p_warm")
            for _ in range(20):
                nc.tensor.matmul(p_warm[:], t_warm[:, :P], t_warm[:],
                                 start=True, stop=True)

            # ============ Phase A: projections ============
            # memT shares a rotation tag with the per-head expT tiles.
            t_memT = sb.tile([P, DC, S], BF16, tag="big_rot", bufs=2,
                             name="t_memT")
            t_wq_c = [sb.tile([P, D], BF16, tag="wqc_eraw", bufs=8,
                                name=f"t_wq_{dc}")
                      for dc in range(DC)]
            # interleave decT/wq chunk loads: the dc-th accumulation matmul
            # only needs chunk dc of each, so the first group starts after
            # just two chunk DMAs instead of the full 4 MB
            for dc in range(DC):
                eng = nc.sync if dc % 2 == 0 else nc.scalar
                eng.dma_start(t_decT_c[dc][:], decT[:, dc, :])
                eng2 = nc.scalar if dc % 2 == 0 else nc.sync
                eng2.dma_start(t_wq_c[dc][:], wq[:, dc, :])
            t_wk = sb.tile([P, DC, D], BF16, tag="w", bufs=2, name="t_wk")

            def proj(dst, w_sl, rhs_sl):
                # dst[:, m, qh] = sum_dc w[dc][:, m-tile].T @ rhs[dc][:, qh]
                for qh in range(2):
                    for m in range(H):
                        pt = ps.tile([P, SH], F32, tag="mm", bufs=2,
                                     name=f"p_mm_{m}_{qh}")
                        for dc in range(DC):
                            nc.tensor.matmul(
                                pt[:],
                                w_sl(dc)[:, m * P:(m + 1) * P],
                                rhs_sl(dc)[:, qh * SH:(qh + 1) * SH],
                                start=(dc == 0), stop=(dc == DC - 1))
                        nc.vector.tensor_copy(
                            dst[:, m, qh * SH:(qh + 1) * SH], pt[:])

            proj(t_qt, lambda dc: t_wq_c[dc][:], lambda dc: t_decT_c[dc][:])

            for dc in range(DC):
                nc.sync.dma_start(t_wk[:, dc, :], wk[:, dc, :])
                nc.scalar.dma_start(t_memT[:, dc, :], memT[:, dc, :])

            for dc in range(DC):
                nc.sync.dma_start(t_keepT[:, dc, :], keepT[:, dc, :])
            nc.sync.dma_start(t_ones_c[:], ones_c[:])
            nc.sync.dma_start(t_qmask[:], qmask[:])

            proj(t_kt, lambda dc: t_wk[:, dc, :], lambda dc: t_memT[:, dc, :])

            t_wv = sb.tile([P, DC, D], BF16, tag="w", bufs=2, name="t_wv")
            for dc in range(DC):
                nc.sync.dma_start(t_wv[:, dc, :], wv[:, dc, :])
            # V native: psum[k-tile, dout-half] = sum_dc memT[:,dc,ktile].T @ wv[:,dc,dh]
            for dh in range(2):
                for kt in range(H):
                    pt = ps.tile([P, SH], F32, tag="mm", bufs=2,
                                 name=f"p_v_{kt}_{dh}")
                    for dc in range(DC):
                        nc.tensor.matmul(
                            pt[:],
                            t_memT[:, dc, kt * P:(kt + 1) * P],
                            t_wv[:, dc, dh * SH:(dh + 1) * SH],
                            start=(dc == 0), stop=(dc == DC - 1))
                    nc.vector.tensor_copy(
                        t_v[:, kt, dh * SH:(dh + 1) * SH], pt[:])

            # ============ Phase B: attention per head ============
            def emit_rep(j, t_rbf, t_rrep, xparts):
                # replicate r across partitions, then scale this head's
                # attn@V psum tiles into xoutT (deferred to overlap with the
                # next head's scores so the PE never stalls on the DVE chain)
                for qh in range(2):
                    pt = ps.tile([P, SH], F32, tag="rep", bufs=1,
                                 name=f"p_rep_{j}_{qh}")
                    nc.tensor.matmul(pt[:], t_ones_r[:],
                                     t_rbf[:, qh * SH:(qh + 1) * SH],
                                     start=True, stop=True)
                    nc.vector.tensor_copy(t_rrep[:, qh * SH:(qh + 1) * SH],
                                          pt[:])
                for qh in range(2):
                    nc.vector.tensor_mul(
                        t_xoutT[:, j, qh * SH:(qh + 1) * SH], xparts[qh],
                        t_rrep[:, qh * SH:(qh + 1) * SH])

            def emit_scores(j):
                t_exp = sb.tile([P, DC, S], BF16, tag="big_rot", bufs=2,
                                name=f"t_exp_{j}")
                for i in range(DC):
                    pt = ps.tile([P, 2, SH], F32, tag="s", bufs=2,
                                 name=f"p_s_{j}_{i}")
                    for qh in range(2):
                        nc.tensor.matmul(
                            pt[:, qh, :],
                            t_kt[:, j, i * P:(i + 1) * P],
                            t_qt[:, j, qh * SH:(qh + 1) * SH],
                            start=True, stop=True)
                    t_eraw = sb.tile([P, S], BF16, tag="wqc_eraw", bufs=8,
                                     name=f"t_eraw_{j}_{i}")
                    nc.scalar.activation(
                        out=t_eraw[:],
                        in_=pt[:].rearrange("p a b -> p (a b)"),
                        func=mybir.ActivationFunctionType.Exp, scale=SCALE)
                    mask_eng = nc.gpsimd if i % 4 == 3 else nc.vector
                    mask_eng.tensor_mul(
                        t_exp[:, i, :], t_eraw[:], t_keepT[:, i, :])
                    nc.sync.dma_start(expt[j, i * P:(i + 1) * P, :],
                                      t_exp[:, i, :])
                return t_exp

            def emit_reduce(j, t_exp):
                # softmax denominators, replicated across partitions via an
                # all-ones [P, P] stationary operand: psum[m, q] = sum_k exp
                t_rrep = sb.tile([P, S], F32, tag="rrep", bufs=1,
                                 name=f"t_rrep_{j}")
                for qh in range(2):
                    pt = ps.tile([P, SH], F32, tag="sum", bufs=2,
                                 name=f"p_sum_{j}_{qh}")
                    for i in range(DC):
                        nc.tensor.matmul(
                            pt[:], t_ones_c[:],
                            t_exp[:, i, qh * SH:(qh + 1) * SH],
                            start=(i == 0), stop=(i == DC - 1))
                    # 1/x as exp(-log(x)) — two fast ACT table ops; the DVE
                    # reciprocal is ~6.4 cycles/element and would pace phase B
                    t_lg = sb.tile([P, SH], F32, tag="lg", bufs=2,
                                   name=f"t_lg_{j}_{qh}")
                    nc.scalar.activation(
                        out=t_lg[:], in_=pt[:],
                        func=mybir.ActivationFunctionType.Ln, scale=1.0)
                    nc.scalar.activation(
                        out=t_rrep[:, qh * SH:(qh + 1) * SH], in_=t_lg[:],
                        func=mybir.ActivationFunctionType.Exp, scale=-1.0)
                nc.vector.tensor_mul(t_rrep[:], t_rrep[:], t_qmask[:])
                nc.sync.dma_start(r_out[j:j + 1, :], t_rrep[0:1, :])

                # attn @ V (transposed): x^T[dv, q] = sum_k V[k, dv] expT[k, q]
                for qh in range(2):
                    pt = ps.tile([P, SH], F32, tag="mm", bufs=2,
                                 name=f"p_x_{j}_{qh}")
                    for i in range(DC):
                        nc.tensor.matmul(
                            pt[:],
                            t_v[:, i, j * P:(j + 1) * P],
                            t_exp[:, i, qh * SH:(qh + 1) * SH],
                            start=(i == 0), stop=(i == DC - 1))
                    nc.vector.tensor_mul(
                        t_xoutT[:, j, qh * SH:(qh + 1) * SH], pt[:],
                        t_rrep[:, qh * SH:(qh + 1) * SH])

            prev = None
            for j in range(H):
                t_exp = emit_scores(j)
                if prev is not None:
                    emit_reduce(prev[0], prev[1])
                prev = (j, t_exp)
            emit_reduce(prev[0], prev[1])

            # ============ Phase C: final linear + residual + LayerNorm ============
            t_wft = sb.tile([P, DC, D], BF16, tag="w", bufs=2, name="t_wft")
            t_wfb = sb.tile([P, DC, D], BF16, tag="w", bufs=2, name="t_wfb")
            for dc in range(DC):
                nc.sync.dma_start(t_wft[:, dc, :], wf_top[:, dc, :])
            for dc in range(DC):
                nc.sync.dma_start(t_wfb[:, dc, :], wf_bot[:, dc, :])

            for t in range(H):
                t_dec = sb.tile([P, D], F32, tag="dec", bufs=2,
                                name=f"t_dec_{t}")
                nc.sync.dma_start(t_dec[:], dec_res[t * P:(t + 1) * P, :])
                t_x2 = sb.tile([P, D], F32, tag="x2", bufs=2, name=f"t_x2_{t}")
                for n in range(2):
                    pt = ps.tile([P, SH], F32, tag="mm", bufs=2,
                                 name=f"p_f_{t}_{n}")
                    for dc in range(DC):
                        nc.tensor.matmul(
                            pt[:],
                            t_decT_c[dc][:, t * P:(t + 1) * P],
                            t_wft[:, dc, n * SH:(n + 1) * SH],
                            start=(dc == 0), stop=False)
                    for j in range(H):
                        nc.tensor.matmul(
                            pt[:],
                            t_xoutT[:, j, t * P:(t + 1) * P],
                            t_wfb[:, j, n * SH:(n + 1) * SH],
                            start=False, stop=(j == H - 1))
                    nc.vector.tensor_add(t_x2[:, n * SH:(n + 1) * SH], pt[:],
                                         t_dec[:, n * SH:(n + 1) * SH])

                # LayerNorm along the free dim (D = 1024, 2 bn_stats subgroups)
                t_stats = sb.tile([P, 2, 6], F32, tag="stats", bufs=2,
                                  name=f"t_stats_{t}")
                for g in range(2):
                    nc.vector.bn_stats(out=t_stats[:, g, :],
                                       in_=t_x2[:, g * SH:(g + 1) * SH])
                t_mv = sb.tile([P, 2], F32, tag="mv", bufs=2, name=f"t_mv_{t}")
                nc.vector.bn_aggr(out=t_mv[:], in_=t_stats[:])
                t_rstd = sb.tile([P, 1], F32, tag="rstd", bufs=2,
                                 name=f"t_rstd_{t}")
                nc.scalar.activation(
                    out=t_rstd[:], in_=t_mv[:, 1:2],
                    func=mybir.ActivationFunctionType.Sqrt,
                    bias=t_eps[:], scale=1.0)
                nc.vector.reciprocal(t_rstd[:], t_rstd[:])
                nc.vector.tensor_scalar(
                    t_x2[:], t_x2[:], scalar1=t_mv[:, 0:1], scalar2=t_rstd[:],
                    op0=mybir.AluOpType.subtract, op1=mybir.AluOpType.mult)
                nc.sync.dma_start(x_out[t * P:(t + 1) * P, :], t_x2[:])

    nc.compile()
    return nc


def _stage_t(arr):
    """[S, D]-like 2D -> [P, DC, S] bf16 with dim0 = (dc, dp) transposed."""
    return np.ascontiguousarray(
        arr.T.reshape(DC, P, -1).transpose(1, 0, 2)).astype(BF)


def _stage_w(w):
    """[D, D] weight -> [P, DC, D] bf16 (contraction rows onto partitions)."""
    return np.ascontiguousarray(
        w.reshape(DC, P, -1).transpose(1, 0, 2)).astype(BF)


def kernel(memory, decoder_input, query_mask, Wk, Wv, Wq, Wf, bf, gamma, beta,
           mask):
    memory = np.asarray(memory, np.float32)
    decoder_input = np.asarray(decoder_input, np.float32)
    query_mask = np.asarray(query_mask, np.float32)
    Wk = np.asarray(Wk, np.float32)
    Wv = np.asarray(Wv, np.float32)
    Wq = np.asarray(Wq, np.float32)
    Wf = np.asarray(Wf, np.float32)
    bf = np.asarray(bf, np.float32)
    gamma = np.asarray(gamma, np.float32)
    beta = np.asarray(beta, np.float32)
    mask = np.asarray(mask)

    if "nc" not in _CACHE:
        _CACHE["nc"] = build()
    nc = _CACHE["nc"]

    shared = dict(
        wq=_stage_w(Wq), wk=_stage_w(Wk), wv=_stage_w(Wv),
        wf_top=_stage_w(Wf[:D]), wf_bot=_stage_w(Wf[D:]),
        ones_c=np.ones((P, P), BF),
    )
    in_maps = []
    for b in range(B):
        in_maps.append(dict(
            shared,
            decT=_stage_t(decoder_input[b]),
            memT=_stage_t(memory[b]),
            keepT=np.ascontiguousarray(
                (~mask[b]).T.astype(np.float32)
                .reshape(DC, P, S).transpose(1, 0, 2)).astype(BF),
            dec_res=np.ascontiguousarray(decoder_input[b] + bf[None, :],
                                         dtype=np.float32),
            qmask=np.ascontiguousarray(
                np.broadcast_to(query_mask[b][None, :], (P, S)),
                dtype=np.float32),
        ))

    res = run_bass_kernel_spmd(nc, in_maps, core_ids=list(range(B)),
                               **_CACHE.get("run_kwargs", {}))
    _CACHE["last_result"] = res

    x = np.empty((B, S, D), np.float32)
    attns = np.empty((H * B, S, S), np.float32)
    apply_gb = (not np.all(gamma == 1.0)) or (not np.all(beta == 0.0))
    for b in range(B):
        rb = res.results[b]
        if apply_gb:
            x[b] = rb["x_out"] * gamma[None, :] + beta[None, :]
        else:
            x[b] = rb["x_out"]
        e = rb["expt"].astype(np.float32)          # [H, k, q]
        r = rb["r_out"]                            # [H, q]
        for j in range(H):
            np.multiply(e[j].T, r[j][:, None], out=attns[j * B + b])
    return x, attns


if __name__ == "__main__":
    rng = np.random.default_rng(0)
    ins = dict(
        memory=rng.standard_normal((B, S, D), dtype=np.float32),
        decoder_input=rng.standard_normal((B, S, D), dtype=np.float32),
        query_mask=rng.random((B, S), dtype=np.float32),
        Wk=(rng.standard_normal((D, D), dtype=np.float32) * 0.02),
        Wv=(rng.standard_normal((D, D), dtype=np.float32) * 0.02),
        Wq=(rng.standard_normal((D, D), dtype=np.float32) * 0.02),
        Wf=(rng.standard_normal((2 * D, D), dtype=np.float32) * 0.02),
        bf=np.zeros(D, np.float32),
        gamma=np.ones(D, np.float32),
        beta=np.zeros(D, np.float32),
        mask=rng.integers(0, 2, (B, S, S)) == 1,
    )
    x, attns = kernel(**ins)
    print("ran", x.shape, attns.shape)


# revision 26
# speedup vs baseline: 1.2767x; 1.0040x over previous
"""Trainium2 Bass kernel for the nn_Attention problem.

Cross-attention transformer block: QKV projections, masked softmax
attention with a post-softmax query-mask multiply, concat + final linear,
residual, LayerNorm.  Returns (x, attns) like the reference.

Sharding: data-parallel over batch B=8 across the 8 NeuronCores — each
core computes one batch element end-to-end; no collectives.

Device-side layout trick: all attention tensors are kept "transposed"
(contraction dim on partitions) so every matmul lhsT/rhs is a natural
slice — the host pre-transposes decoder/memory/mask per core, and the
attention weights output is produced as expT [k, q] + row-normalizers r,
with the final attns = (expT.T * r) assembled on the host.
"""
import sys, os

for p in ("/opt/trn_rl_repo",):
    if p not in sys.path and os.path.isdir(p):
        sys.path.insert(0, p)

import numpy as np
import ml_dtypes

import concourse.bass as bass
import concourse.mybir as mybir
import concourse.tile as tile
from concourse import bacc
from concourse.bass_utils import run_bass_kernel_spmd

F32 = mybir.dt.float32
BF16 = mybir.dt.bfloat16
BF = ml_dtypes.bfloat16

B, S, D, H, P = 8, 1024, 1024, 8, 128
DC = D // P          # 8 contraction chunks of 128
SH = 512             # free-dim half
NEG = float(-2**32 + 1)
SCALE = float(1.0 / np.sqrt(P))  # 1/sqrt(dh), dh = 128
LN_EPS = 1e-5

_CACHE = {}


def _patch_act_tables():
    """Steer the ACT table-set chooser to the combined ln+exp set so the
    per-head 1/x = exp(-ln(x)) pair doesn't ping-pong table loads
    (~1.3 us per swap, 2 swaps per head otherwise)."""
    import concourse.hw_specs as hw_specs
    orig = hw_specs.get_activation_tables
    EXP = mybir.ActivationFunctionType.Exp
    LN = mybir.ActivationFunctionType.Ln

    def patched(module_arch):
        tables = orig(module_arch)
        for name, fns in tables.items():
            if name != "natural_log_exp_and_others":
                fns.discard(EXP)
                fns.discard(LN)
        return tables

    bacc.get_activation_tables = patched


def build():
    _patch_act_tables()
    nc = bacc.Bacc(None, target_bir_lowering=False)

    # ---- per-core inputs (bf16 matmul operands staged by host) ----
    decT = nc.dram_tensor("decT", [P, DC, S], BF16, kind="ExternalInput")
    memT = nc.dram_tensor("memT", [P, DC, S], BF16, kind="ExternalInput")
    keepT = nc.dram_tensor("keepT", [P, DC, S], BF16, kind="ExternalInput")
    dec_res = nc.dram_tensor("dec_res", [S, D], F32, kind="ExternalInput")
    qmask = nc.dram_tensor("qmask", [P, S], F32, kind="ExternalInput")
    # ---- shared weights / constants ----
    wq = nc.dram_tensor("wq", [P, DC, D], BF16, kind="ExternalInput")
    wk = nc.dram_tensor("wk", [P, DC, D], BF16, kind="ExternalInput")
    wv = nc.dram_tensor("wv", [P, DC, D], BF16, kind="ExternalInput")
    wf_top = nc.dram_tensor("wf_top", [P, DC, D], BF16, kind="ExternalInput")
    wf_bot = nc.dram_tensor("wf_bot", [P, DC, D], BF16, kind="ExternalInput")
    ones_c = nc.dram_tensor("ones_c", [P, P], BF16, kind="ExternalInput")

    # ---- outputs ----
    x_out = nc.dram_tensor("x_out", [S, D], F32, kind="ExternalOutput")
    expt = nc.dram_tensor("expt", [H, S, S], BF16, kind="ExternalOutput")
    r_out = nc.dram_tensor("r_out", [H, S], F32, kind="ExternalOutput")

    with tile.TileContext(nc) as tc:
        with (
            tc.tile_pool(name="sb", bufs=1) as sb,
            tc.tile_pool(name="ps", bufs=1, space="PSUM") as ps,
        ):
            # ---------- constants ----------
            t_ones_c = sb.tile([P, P], BF16)
            t_qmask = sb.tile([P, S], F32)
            t_eps = sb.tile([P, 1], F32)
            nc.vector.memset(t_eps[:], LN_EPS)

            # ---------- persistent big tiles ----------
            # decT/wq as per-chunk tiles so the first projection matmuls can
            # start as soon as chunk 0 lands (deps are tracked per tile)
            t_decT_c = [sb.tile([P, S], BF16, name=f"t_decT_{dc}")
                        for dc in range(DC)]
            t_keepT = sb.tile([P, DC, S], BF16)
            # (keepT DMA deferred below so early-phase DMAs get the bandwidth)
            t_qt = sb.tile([P, H, S], BF16)      # Q^T  [dh, head, q]
            t_kt = sb.tile([P, H, S], BF16)      # K^T  [dh, head, k]
            t_v = sb.tile([P, DC, D], BF16)      # V    [k_inner, k_chunk, dout]
            t_xoutT = sb.tile([P, H, S], BF16)   # attn_out^T [dh, head, q]


            # ---------- PE warm-up ----------
            # ~20 throwaway matmuls keep the PE HAM activity monitor busy
            # while the first input DMAs land, so real matmuls start at the
            # full 2.4 GHz clock instead of the cold 1.2 GHz state.
            t_warm = sb.tile([P, SH], BF16)
            nc.vector.memset(t_warm[:], 0.0)
            p_warm = ps.tile([P, SH], F32, tag="mm", bufs=3, name="p_warm")
            for _ in range(32):
                nc.tensor.matmul(p_warm[:], t_warm[:, :P], t_warm[:],
                                 start=True, stop=True)

            # ============ Phase A: projections ============
            # memT shares a rotation tag with the per-head expT tiles.
            t_memT = sb.tile([P, DC, S], BF16, tag="big_rot", bufs=2,
                             name="t_memT")
            t_wq_c = [sb.tile([P, D], BF16, tag="wqc_eraw", bufs=8,
                                name=f"t_wq_{dc}")
                      for dc in range(DC)]
            # interleave decT/wq chunk loads: the dc-th accumulation matmul
            # only needs chunk dc of each, so the first group starts after
            # just two chunk DMAs instead of the full 4 MB
            for dc in range(DC):
                eng = nc.sync if dc % 2 == 0 else nc.scalar
                eng.dma_start(t_decT_c[dc][:], decT[:, dc, :])
                eng2 = nc.scalar if dc % 2 == 0 else nc.sync
                eng2.dma_start(t_wq_c[dc][:], wq[:, dc, :])
            t_wk = sb.tile([P, DC, D], BF16, tag="w", bufs=2, name="t_wk")

            def proj(dst, w_sl, rhs_sl):
                # dst[:, m, qh] = sum_dc w[dc][:, m-tile].T @ rhs[dc][:, qh]
                for qh in range(2):
                    for m in range(H):
                        pt = ps.tile([P, SH], F32, tag="mm", bufs=3,
                                     name=f"p_mm_{m}_{qh}")
                        for dc in range(DC):
                            nc.tensor.matmul(
                                pt[:],
                                w_sl(dc)[:, m * P:(m + 1) * P],
                                rhs_sl(dc)[:, qh * SH:(qh + 1) * SH],
                                start=(dc == 0), stop=(dc == DC - 1))
                        nc.vector.tensor_copy(
                            dst[:, m, qh * SH:(qh + 1) * SH], pt[:])

            proj(t_qt, lambda dc: t_wq_c[dc][:], lambda dc: t_decT_c[dc][:])

            for dc in range(DC):
                nc.sync.dma_start(t_wk[:, dc, :], wk[:, dc, :])
                nc.scalar.dma_start(t_memT[:, dc, :], memT[:, dc, :])

            for dc in range(DC):
                nc.sync.dma_start(t_keepT[:, dc, :], keepT[:, dc, :])
            nc.sync.dma_start(t_ones_c[:], ones_c[:])
            nc.sync.dma_start(t_qmask[:], qmask[:])

            proj(t_kt, lambda dc: t_wk[:, dc, :], lambda dc: t_memT[:, dc, :])

            t_wv = sb.tile([P, DC, D], BF16, tag="w", bufs=2, name="t_wv")
            for dc in range(DC):
                nc.sync.dma_start(t_wv[:, dc, :], wv[:, dc, :])
            # V native: psum[k-tile, dout-half] = sum_dc memT[:,dc,ktile].T @ wv[:,dc,dh]
            for dh in range(2):
                for kt in range(H):
                    pt = ps.tile([P, SH], F32, tag="mm", bufs=3,
                                 name=f"p_v_{kt}_{dh}")
                    for dc in range(DC):
                        nc.tensor.matmul(
                            pt[:],
                            t_memT[:, dc, kt * P:(kt + 1) * P],
                            t_wv[:, dc, dh * SH:(dh + 1) * SH],
                            start=(dc == 0), stop=(dc == DC - 1))
                    nc.vector.tensor_copy(
                        t_v[:, kt, dh * SH:(dh + 1) * SH], pt[:])

            # ============ Phase B: attention per head ============
            def emit_rep(j, t_rbf, t_rrep, xparts):
                # replicate r across partitions, then scale this head's
                # attn@V psum tiles into xoutT (deferred to overlap with the
                # next head's scores so the PE never stalls on the DVE chain)
                for qh in range(2):
                    pt = ps.tile([P, SH], F32, tag="rep", bufs=1,
                                 name=f"p_rep_{j}_{qh}")
                    nc.tensor.matmul(pt[:], t_ones_r[:],
                                     t_rbf[:, qh * SH:(qh + 1) * SH],
                                     start=True, stop=True)
                    nc.vector.tensor_copy(t_rrep[:, qh * SH:(qh + 1) * SH],
                                          pt[:])
                for qh in range(2):
                    nc.vector.tensor_mul(
                        t_xoutT[:, j, qh * SH:(qh + 1) * SH], xparts[qh],
                        t_rrep[:, qh * SH:(qh + 1) * SH])

            def emit_scores(j):
                t_exp = sb.tile([P, DC, S], BF16, tag="big_rot", bufs=2,
                                name=f"t_exp_{j}")
                for i in range(DC):
                    pt = ps.tile([P, 2, SH], F32, tag="s", bufs=2,
                                 name=f"p_s_{j}_{i}")
                    for qh in range(2):
                        nc.tensor.matmul(
                            pt[:, qh, :],
                            t_kt[:, j, i * P:(i + 1) * P],
                            t_qt[:, j, qh * SH:(qh + 1) * SH],
                            start=True, stop=True)
                    t_eraw = sb.tile([P, S], BF16, tag="wqc_eraw", bufs=8,
                                     name=f"t_eraw_{j}_{i}")
                    nc.scalar.activation(
                        out=t_eraw[:],
                        in_=pt[:].rearrange("p a b -> p (a b)"),
                        func=mybir.ActivationFunctionType.Exp, scale=SCALE)
                    mask_eng = nc.gpsimd if i % 4 == 3 else nc.vector
                    mask_eng.tensor_mul(
                        t_exp[:, i, :], t_eraw[:], t_keepT[:, i, :])
                    nc.sync.dma_start(expt[j, i * P:(i + 1) * P, :],
                                      t_exp[:, i, :])
                return t_exp

            def emit_reduce(j, t_exp):
                # softmax denominators, replicated across partitions via an
                # all-ones [P, P] stationary operand: psum[m, q] = sum_k exp
                t_rrep = sb.tile([P, S], F32, tag="rrep", bufs=1,
                                 name=f"t_rrep_{j}")
                for qh in range(2):
                    pt = ps.tile([P, SH], F32, tag="sum", bufs=1,
                                 name=f"p_sum_{j}_{qh}")
                    for i in range(DC):
                        nc.tensor.matmul(
                            pt[:], t_ones_c[:],
                            t_exp[:, i, qh * SH:(qh + 1) * SH],
                            start=(i == 0), stop=(i == DC - 1))
                    # 1/x as exp(-log(x)) — two fast ACT table ops; the DVE
                    # reciprocal is ~6.4 cycles/element and would pace phase B
                    t_lg = sb.tile([P, SH], F32, tag="lg", bufs=2,
                                   name=f"t_lg_{j}_{qh}")
                    nc.scalar.activation(
                        out=t_lg[:], in_=pt[:],
                        func=mybir.ActivationFunctionType.Ln, scale=1.0)
                    nc.scalar.activation(
                        out=t_rrep[:, qh * SH:(qh + 1) * SH], in_=t_lg[:],
                        func=mybir.ActivationFunctionType.Exp, scale=-1.0)
                nc.vector.tensor_mul(t_rrep[:], t_rrep[:], t_qmask[:])
                nc.sync.dma_start(r_out[j:j + 1, :], t_rrep[0:1, :])

                # attn @ V (transposed): x^T[dv, q] = sum_k V[k, dv] expT[k, q]
                for qh in range(2):
                    pt = ps.tile([P, SH], F32, tag="mm", bufs=3,
                                 name=f"p_x_{j}_{qh}")
                    for i in range(DC):
                        nc.tensor.matmul(
                            pt[:],
                            t_v[:, i, j * P:(j + 1) * P],
                            t_exp[:, i, qh * SH:(qh + 1) * SH],
                            start=(i == 0), stop=(i == DC - 1))
                    nc.vector.tensor_mul(
                        t_xoutT[:, j, qh * SH:(qh + 1) * SH], pt[:],
                        t_rrep[:, qh * SH:(qh + 1) * SH])

            prev = None
            for j in range(H):
                t_exp = emit_scores(j)
                if prev is not None:
                    emit_reduce(prev[0], prev[1])
                prev = (j, t_exp)
            emit_reduce(prev[0], prev[1])

            # ============ Phase C: final linear + residual + LayerNorm ============
            t_wft = sb.tile([P, DC, D], BF16, tag="w", bufs=2, name="t_wft")
            t_wfb = sb.tile([P, DC, D], BF16, tag="w", bufs=2, name="t_wfb")
            for dc in range(DC):
                nc.sync.dma_start(t_wft[:, dc, :], wf_top[:, dc, :])
            for dc in range(DC):
                nc.sync.dma_start(t_wfb[:, dc, :], wf_bot[:, dc, :])

            for t in range(H):
                t_dec = sb.tile([P, D], F32, tag="dec", bufs=2,
                                name=f"t_dec_{t}")
                nc.sync.dma_start(t_dec[:], dec_res[t * P:(t + 1) * P, :])
                t_x2 = sb.tile([P, D], F32, tag="x2", bufs=2, name=f"t_x2_{t}")
                for n in range(2):
                    pt = ps.tile([P, SH], F32, tag="mm", bufs=3,
                                 name=f"p_f_{t}_{n}")
                    for dc in range(DC):
                        nc.tensor.matmul(
                            pt[:],
                            t_decT_c[dc][:, t * P:(t + 1) * P],
                            t_wft[:, dc, n * SH:(n + 1) * SH],
                            start=(dc == 0), stop=False)
                    for j in range(H):
                        nc.tensor.matmul(
                            pt[:],
                            t_xoutT[:, j, t * P:(t + 1) * P],
                            t_wfb[:, j, n * SH:(n + 1) * SH],
                            start=False, stop=(j == H - 1))
                    nc.vector.tensor_add(t_x2[:, n * SH:(n + 1) * SH], pt[:],
                                         t_dec[:, n * SH:(n + 1) * SH])

                # LayerNorm along the free dim (D = 1024, 2 bn_stats subgroups)
                t_stats = sb.tile([P, 2, 6], F32, tag="stats", bufs=2,
                                  name=f"t_stats_{t}")
                for g in range(2):
                    nc.vector.bn_stats(out=t_stats[:, g, :],
                                       in_=t_x2[:, g * SH:(g + 1) * SH])
                t_mv = sb.tile([P, 2], F32, tag="mv", bufs=2, name=f"t_mv_{t}")
                nc.vector.bn_aggr(out=t_mv[:], in_=t_stats[:])
                t_rstd = sb.tile([P, 1], F32, tag="rstd", bufs=2,
                                 name=f"t_rstd_{t}")
                nc.scalar.activation(
                    out=t_rstd[:], in_=t_mv[:, 1:2],
                    func=mybir.ActivationFunctionType.Sqrt,
                    bias=t_eps[:], scale=1.0)
                nc.vector.reciprocal(t_rstd[:], t_rstd[:])
                nc.vector.tensor_scalar(
                    t_x2[:], t_x2[:], scalar1=t_mv[:, 0:1], scalar2=t_rstd[:],
                    op0=mybir.AluOpType.subtract, op1=mybir.AluOpType.mult)
                nc.sync.dma_start(x_out[t * P:(t + 1) * P, :], t_x2[:])

    nc.compile()
    return nc


def _stage_t(arr):
    """[S, D]-like 2D -> [P, DC, S] bf16 with dim0 = (dc, dp) transposed."""
    return np.ascontiguousarray(
        arr.T.reshape(DC, P, -1).transpose(1, 0, 2)).astype(BF)


def _stage_w(w):
    """[D, D] weight -> [P, DC, D] bf16 (contraction rows onto partitions)."""
    return np.ascontiguousarray(
        w.reshape(DC, P, -1).transpose(1, 0, 2)).astype(BF)


def kernel(memory, decoder_input, query_mask, Wk, Wv, Wq, Wf, bf, gamma, beta,
           mask):
    memory = np.asarray(memory, np.float32)
    decoder_input = np.asarray(decoder_input, np.float32)
    query_mask = np.asarray(query_mask, np.float32)
    Wk = np.asarray(Wk, np.float32)
    Wv = np.asarray(Wv, np.float32)
    Wq = np.asarray(Wq, np.float32)
    Wf = np.asarray(Wf, np.float32)
    bf = np.asarray(bf, np.float32)
    gamma = np.asarray(gamma, np.float32)
    beta = np.asarray(beta, np.float32)
    mask = np.asarray(mask)

    if "nc" not in _CACHE:
        _CACHE["nc"] = build()
    nc = _CACHE["nc"]

    shared = dict(
        wq=_stage_w(Wq), wk=_stage_w(Wk), wv=_stage_w(Wv),
        wf_top=_stage_w(Wf[:D]), wf_bot=_stage_w(Wf[D:]),
        ones_c=np.ones((P, P), BF),
    )
    in_maps = []
    for b in range(B):
        in_maps.append(dict(
            shared,
            decT=_stage_t(decoder_input[b]),
            memT=_stage_t(memory[b]),
            keepT=np.ascontiguousarray(
                (~mask[b]).T.astype(np.float32)
                .reshape(DC, P, S).transpose(1, 0, 2)).astype(BF),
            dec_res=np.ascontiguousarray(decoder_input[b] + bf[None, :],
                                         dtype=np.float32),
            qmask=np.ascontiguousarray(
                np.broadcast_to(query_mask[b][None, :], (P, S)),
                dtype=np.float32),
        ))

    res = run_bass_kernel_spmd(nc, in_maps, core_ids=list(range(B)),
                               **_CACHE.get("run_kwargs", {}))
    _CACHE["last_result"] = res

    x = np.empty((B, S, D), np.float32)
    attns = np.empty((H * B, S, S), np.float32)
    apply_gb = (not np.all(gamma == 1.0)) or (not np.all(beta == 0.0))
    for b in range(B):
        rb = res.results[b]
        if apply_gb:
            x[b] = rb["x_out"] * gamma[None, :] + beta[None, :]
        else:
            x[b] = rb["x_out"]
        e = rb["expt"].astype(np.float32)          # [H, k, q]
        r = rb["r_out"]                            # [H, q]
        for j in range(H):
            np.multiply(e[j].T, r[j][:, None], out=attns[j * B + b])
    return x, attns


if __name__ == "__main__":
    rng = np.random.default_rng(0)
    ins = dict(
        memory=rng.standard_normal((B, S, D), dtype=np.float32),
        decoder_input=rng.standard_normal((B, S, D), dtype=np.float32),
        query_mask=rng.random((B, S), dtype=np.float32),
        Wk=(rng.standard_normal((D, D), dtype=np.float32) * 0.02),
        Wv=(rng.standard_normal((D, D), dtype=np.float32) * 0.02),
        Wq=(rng.standard_normal((D, D), dtype=np.float32) * 0.02),
        Wf=(rng.standard_normal((2 * D, D), dtype=np.float32) * 0.02),
        bf=np.zeros(D, np.float32),
        gamma=np.ones(D, np.float32),
        beta=np.zeros(D, np.float32),
        mask=rng.integers(0, 2, (B, S, S)) == 1,
    )
    x, attns = kernel(**ins)
    print("ran", x.shape, attns.shape)
